# revision 11
# baseline (speedup 1.0000x reference)
"""GNN message-passing kernel for 8 Trainium2 NeuronCores.

Strategy (destination-sharded, degree-sorted):
  - Nodes are assigned to cores round-robin by degree rank, so every core's
    per-degree node counts match (after tiny padding) and one SPMD program
    serves all 8 cores.
  - Each core's edges are laid out grouped by destination node, nodes grouped
    by exact degree d.  The segment-sum over edges then becomes a dense
    [128ch, n_nodes, d] free-axis reduction - no scatter, no one-hot.
  - Edge MLP runs channels-on-partitions: in^T [43, E] tiles, two stationary
    weight matmuls (43->64, 65->128 with bias folded via a ones row).
  - Node MLP runs on the mean slab [128ch, node_slots] directly.
  - Host does index preprocessing, the x[col] gather into the edge-feature
    shard, and the final unpermute.
"""

import math
import os

import numpy as np
import ml_dtypes

N_NODES = 100000
N_EDGES = 1600000
N_CORES = 8
X_DIM, E_DIM = 4, 39
EAX_DIM = X_DIM + E_DIM  # 43
H1, H2 = 64, 128
H3 = 256

ROUND_EDGES = 512  # compute-round edge budget (one PSUM bank at fp32)
DMA_EDGES = 4096   # edge-feature DMA granularity

RUN_KWARGS: dict = {}
LAST_EXEC_NS = None
LAST_RESULT = None

F32 = "float32"


def _apply_tile_drain_patch():
    """walrus in this env only accepts one sync wait on a TPB_CTRL drain;
    split the Tile tail drain's waits across multiple drain instructions."""
    import bass_rust
    from concourse.tile import TileContext, ScopedClock

    if getattr(TileContext, "_drain_patch_applied", False):
        return

    def _patched(self, tick_clock, wait_clock):
        nc = self.nc
        drain_inst = nc.sync.drain()
        wait_clock.add_sem_waits(
            drain_inst.ins, ScopedClock({None: tick_clock.global_clock})
        )
        si = drain_inst.ins.sync_info
        waits = list(si.on_wait) if si is not None else []
        if len(waits) > 1:
            drain_inst.ins.sync_info = bass_rust.SyncInfo(
                on_wait=[waits[0]], on_update=[]
            )
            for w in waits[1:]:
                d2 = nc.sync.drain()
                d2.ins.sync_info = bass_rust.SyncInfo(on_wait=[w], on_update=[])
        nc.all_engine_barrier()
        assert self.sems is not None
        popped = nc._tile_sem_poison_stack.pop()
        assert popped is self._sem_poison
        nc.clear_and_free_semaphores(list(self.sems.allocated().values()))
        nc.all_engine_barrier()

    TileContext._drain_and_barrier = _patched
    TileContext._drain_patch_applied = True


# ---------------------------------------------------------------------------
# Host-side preprocessing
# ---------------------------------------------------------------------------

def _preprocess(x, row, col, edge_attr, edge_dt, node_dt):
    """Build per-core shards. Returns (in_maps, meta)."""
    deg = np.bincount(row, minlength=N_NODES).astype(np.int64)

    # Degree-ascending node order; node i of the order goes to core i % 8.
    node_order = np.argsort(deg, kind="stable")
    deg_sorted = deg[node_order]

    # Per-degree uniform region sizes M_d = max over cores of per-core count.
    degrees = np.unique(deg_sorted)
    # count of nodes with degree d on core c: split counts of each degree run
    # over cores: run of length L starting at global index s -> core (s+k)%8.
    run_starts = np.searchsorted(deg_sorted, degrees, side="left")
    run_lens = np.searchsorted(deg_sorted, degrees, side="right") - run_starts
    # M_d: ceil division accounting for phase; max over cores is
    # ceil(L/8) when L%8 != 0 aligned anywhere -> just use ceil(L/8) if the
    # run is spread evenly, but phase can make one core get one extra:
    # max count = ceil((L + (s % 8 accounted)) ... simply compute exactly.
    M = np.empty(len(degrees), dtype=np.int64)
    m_dc = np.empty((len(degrees), N_CORES), dtype=np.int64)
    for j, (s, L) in enumerate(zip(run_starts, run_lens)):
        idx = (s + np.arange(L)) % N_CORES
        cnt = np.bincount(idx, minlength=N_CORES)
        m_dc[j] = cnt
        M[j] = cnt.max()

    node_off = np.concatenate([[0], np.cumsum(M)])  # region node-slot offsets
    NS = int(node_off[-1])  # node slots per core (incl. per-degree pads)
    NS_pad = ((NS + 511) // 512) * 512
    edge_off = np.concatenate([[0], np.cumsum(M * degrees)])
    E_TOT = int(edge_off[-1])
    E_TOT_pad = ((E_TOT + DMA_EDGES - 1) // DMA_EDGES) * DMA_EDGES

    # --- per-core slot assignment -----------------------------------------
    # nodes of core c in degree order: node_order[c::8] with degree run
    # boundaries; slot of k-th node of degree d on core c = node_off[j] + k.
    in_maps = []
    slot_tables = []  # per core: global node id per slot (-1 pad)
    x = x.astype(np.float32)
    edge_attr = edge_attr.astype(np.float32)
    recip = 1.0 / np.maximum(deg, 1.0)

    # edge -> (core, slot) of its destination
    # global: position of node in sorted order
    pos_of_node = np.empty(N_NODES, dtype=np.int64)
    pos_of_node[node_order] = np.arange(N_NODES)
    core_of_node = pos_of_node % N_CORES

    # per-core, per-degree start index within the core's degree-sorted list
    for c in range(N_CORES):
        nodes_c = node_order[c::N_CORES]  # ascending degree
        deg_c = deg[nodes_c]
        # within-degree rank for this core's nodes
        # nodes_c sorted by degree; run starts:
        starts_c = np.searchsorted(deg_c, degrees, side="left")
        j_of_deg = np.searchsorted(degrees, deg_c)  # region index per node
        rank = np.arange(len(nodes_c)) - starts_c[j_of_deg]
        slot = node_off[j_of_deg] + rank

        slots = np.full(NS_pad, -1, dtype=np.int64)
        slots[slot] = nodes_c
        slot_tables.append(slots)

        # --- edges of this core ------------------------------------------
        emask = core_of_node[row] == c
        er = row[emask]
        ec = col[emask]
        ea = edge_attr[emask]
        # slot of dest node
        slot_of_node = np.full(N_NODES, -1, dtype=np.int64)
        slot_of_node[nodes_c] = slot
        es = slot_of_node[er]
        order = np.argsort(es, kind="stable")
        es_s = es[order]
        # within-node running index
        uniq, first_idx, counts = np.unique(
            es_s, return_index=True, return_counts=True
        )
        within = np.arange(len(es_s)) - np.repeat(first_idx, counts)
        # edge slot base per node slot: edge_off[j] + (slot-node_off[j])*d
        j_of_slot = j_of_deg[np.argsort(slot, kind="stable")]  # slot-> region
        # simpler: recompute region of each sorted edge's dest slot
        j_e = np.searchsorted(node_off[1:], es_s, side="right")
        d_e = degrees[j_e]
        ebase = edge_off[j_e] + (es_s - node_off[j_e]) * d_e
        epos = ebase + within

        eax = np.zeros((EAX_DIM, E_TOT_pad), dtype=np.float32)
        eax[:E_DIM, epos] = ea[order].T
        eax[E_DIM:, epos] = x[ec[order]].T

        xT = np.zeros((X_DIM, NS_pad), dtype=np.float32)
        valid = slots >= 0
        xT[:, valid] = x[slots[valid]].T
        rr = np.ones(NS_pad, dtype=np.float32)
        rr[valid] = recip[slots[valid]]
        rrep = np.broadcast_to(rr[None, :], (128, NS_pad)).copy()

        in_maps.append(
            {
                "eax": eax.astype(edge_dt),
                "xT": xT.astype(node_dt),
                "rrep": rrep,
            }
        )

    meta = {
        "degrees": degrees.tolist(),
        "M": M.tolist(),
        "node_off": node_off.tolist(),
        "edge_off": edge_off.tolist(),
        "NS": NS,
        "NS_pad": NS_pad,
        "E_TOT": E_TOT,
        "E_TOT_pad": E_TOT_pad,
        "slot_tables": slot_tables,
    }
    return in_maps, meta


# ---------------------------------------------------------------------------
# Device program
# ---------------------------------------------------------------------------

def _build_program(meta, W1, b1, W2, b2, W3, b3, W4, b4, edge_dt, node_dt,
                   matmul_f32r=True):
    import concourse.bass as bass
    import concourse.mybir as mybir
    from concourse import bacc
    from concourse.tile import TileContext

    _apply_tile_drain_patch()

    DT_E = mybir.dt.bfloat16 if edge_dt == ml_dtypes.bfloat16 else mybir.dt.float32
    DT_N = mybir.dt.bfloat16 if node_dt == ml_dtypes.bfloat16 else mybir.dt.float32
    f32 = mybir.dt.float32

    NS_pad = meta["NS_pad"]
    E_TOT_pad = meta["E_TOT_pad"]
    degrees = meta["degrees"]
    M = meta["M"]
    node_off = meta["node_off"]
    edge_off = meta["edge_off"]

    nc = bacc.Bacc("TRN2", target_bir_lowering=False, debug=False)
    eax = nc.declare_dram_parameter("eax", [EAX_DIM, E_TOT_pad], DT_E, isOutput=False)
    xT = nc.declare_dram_parameter("xT", [X_DIM, NS_pad], DT_N, isOutput=False)
    rrep = nc.declare_dram_parameter("rrep", [128, NS_pad], f32, isOutput=False)
    w1c = nc.declare_dram_parameter("w1c", [EAX_DIM, H1], DT_E, isOutput=False)
    b1d = nc.declare_dram_parameter("b1d", [H1, 1], f32, isOutput=False)
    w2a = nc.declare_dram_parameter("w2a", [H1 + 1, H2], DT_E, isOutput=False)
    w3m = nc.declare_dram_parameter("w3m", [H2, H3], DT_N, isOutput=False)
    w3x = nc.declare_dram_parameter("w3x", [X_DIM, H3], DT_N, isOutput=False)
    w4a = nc.declare_dram_parameter("w4a", [128, H3], DT_N, isOutput=False)
    w4b = nc.declare_dram_parameter("w4b", [128, H3], DT_N, isOutput=False)
    b3d = nc.declare_dram_parameter("b3d", [128, 2], f32, isOutput=False)
    b4d = nc.declare_dram_parameter("b4d", [128, 2], f32, isOutput=False)
    oT = nc.declare_dram_parameter("oT", [H3, NS_pad], f32, isOutput=True)

    def mmcast(ap):
        if matmul_f32r and ap.dtype == mybir.dt.float32:
            return ap.bitcast(mybir.dt.float32r)
        return ap

    # build edge-round schedule: list of (dma ranges) and rounds
    # round: (edge_start, n_nodes, d, node_slot_start) all within one region
    rounds = []
    for j, d in enumerate(degrees):
        if d == 0:
            continue
        m = M[j]
        npr = max(1, ROUND_EDGES // d)  # nodes per round
        s = 0
        while s < m:
            n = min(npr, m - s)
            rounds.append((edge_off[j] + s * d, n, d, node_off[j] + s))
            s += n
    # pack rounds into DMA chunks of <= DMA_EDGES contiguous edges
    chunks = []  # (dma_start, dma_len, [round indices])
    cur = None
    for ri, (e0, n, d, s0) in enumerate(rounds):
        ln = n * d
        if cur is not None and e0 + ln - cur[0] <= DMA_EDGES:
            cur[1] = e0 + ln - cur[0]
            cur[2].append(ri)
        else:
            if cur is not None:
                chunks.append(tuple(cur))
            cur = [e0, ln, [ri]]
    if cur is not None:
        chunks.append(tuple(cur))

    with TileContext(nc) as tc:
      with tc.tile_pool(name="slab", bufs=1) as slabp:
        mean_slab = slabp.tile([128, NS_pad], f32)
        with (
            tc.tile_pool(name="const", bufs=1) as constp,
            tc.tile_pool(name="eaxp", bufs=3) as eaxp,
            tc.tile_pool(name="h1p", bufs=3) as h1p,
            tc.tile_pool(name="h2p", bufs=3) as h2p,
            tc.tile_pool(name="ph1p", bufs=3, space="PSUM") as ph1p,
            tc.tile_pool(name="ph2p", bufs=3, space="PSUM") as ph2p,
        ):
            w1c_sb = constp.tile([EAX_DIM, H1], DT_E)
            nc.gpsimd.dma_start(out=w1c_sb[:], in_=w1c[:])
            w2a_sb = constp.tile([H1 + 1, H2], DT_E)
            nc.gpsimd.dma_start(out=w2a_sb[:], in_=w2a[:])
            b1_sb = constp.tile([H1, 1], f32)
            nc.gpsimd.dma_start(out=b1_sb[:], in_=b1d[:])

            nc.gpsimd.memset(mean_slab[:], 0.0)

            for (c0, clen, ris) in chunks:
                eax_t = eaxp.tile([EAX_DIM, DMA_EDGES], DT_E, tag="eax")
                nc.sync.dma_start(out=eax_t[:, :clen], in_=eax[:, c0 : c0 + clen])
                for ri in ris:
                    e0, n, d, s0 = rounds[ri]
                    re = n * d
                    ro = e0 - c0
                    ph1 = ph1p.tile([H1, ROUND_EDGES], f32, tag="ph1")
                    nc.tensor.matmul(
                        out=ph1[:, :re],
                        lhsT=mmcast(w1c_sb[:]),
                        rhs=mmcast(eax_t[:, ro : ro + re]),
                        start=True,
                        stop=True,
                    )
                    h1t = h1p.tile([H1 + 1, ROUND_EDGES], DT_E, tag="h1")
                    nc.scalar.activation(
                        out=h1t[:H1, :re],
                        in_=ph1[:, :re],
                        func=mybir.ActivationFunctionType.Relu,
                        bias=b1_sb[:],
                    )
                    nc.gpsimd.memset(h1t[H1 : H1 + 1, :re], 1.0)
                    ph2 = ph2p.tile([H2, ROUND_EDGES], f32, tag="ph2")
                    nc.tensor.matmul(
                        out=ph2[:, :re],
                        lhsT=mmcast(w2a_sb[:]),
                        rhs=mmcast(h1t[:, :re]),
                        start=True,
                        stop=True,
                    )
                    h2t = h2p.tile([H2, ROUND_EDGES], DT_E, tag="h2")
                    nc.scalar.activation(
                        out=h2t[:, :re],
                        in_=ph2[:, :re],
                        func=mybir.ActivationFunctionType.Relu,
                    )
                    if d <= ROUND_EDGES:
                        src = h2t[:, :re].rearrange("p (n d) -> p n d", d=d)
                        nc.vector.tensor_reduce(
                            out=mean_slab[:, s0 : s0 + n],
                            in_=src,
                            op=mybir.AluOpType.add,
                            axis=mybir.AxisListType.X,
                        )
                    else:
                        raise NotImplementedError("degree > ROUND_EDGES")

            # mean scaling by 1/deg
            rrep_sb = constp.tile([128, NS_pad], f32)
            nc.gpsimd.dma_start(out=rrep_sb[:], in_=rrep[:])
            for s in range(0, NS_pad, 512):
                nc.vector.tensor_tensor(
                    out=mean_slab[:, s : s + 512],
                    in0=mean_slab[:, s : s + 512],
                    in1=rrep_sb[:, s : s + 512],
                    op=mybir.AluOpType.mult,
                )

        # ---- node MLP phase ----
        with (
            tc.tile_pool(name="nconst", bufs=1) as nconstp,
            tc.tile_pool(name="o1p", bufs=4) as o1p,
            tc.tile_pool(name="o2p", bufs=3) as o2p,
            tc.tile_pool(name="nps", bufs=4, space="PSUM") as nps,
        ):
            w3m_sb = nconstp.tile([H2, H3], DT_N)
            nc.gpsimd.dma_start(out=w3m_sb[:], in_=w3m[:])
            w3x_sb = nconstp.tile([X_DIM, H3], DT_N)
            nc.gpsimd.dma_start(out=w3x_sb[:], in_=w3x[:])
            w4a_sb = nconstp.tile([128, H3], DT_N)
            nc.gpsimd.dma_start(out=w4a_sb[:], in_=w4a[:])
            w4b_sb = nconstp.tile([128, H3], DT_N)
            nc.gpsimd.dma_start(out=w4b_sb[:], in_=w4b[:])
            b3_sb = nconstp.tile([128, 2], f32)
            nc.gpsimd.dma_start(out=b3_sb[:], in_=b3d[:])
            b4_sb = nconstp.tile([128, 2], f32)
            nc.gpsimd.dma_start(out=b4_sb[:], in_=b4d[:])
            xT_sb = nconstp.tile([X_DIM, NS_pad], DT_N)
            nc.gpsimd.dma_start(out=xT_sb[:], in_=xT[:])

            # mean slab may need dtype cast for bf16 node matmuls
            if DT_N == mybir.dt.bfloat16:
                mean_n = nconstp.tile([128, NS_pad], DT_N)
                for s in range(0, NS_pad, 2048):
                    e = min(s + 2048, NS_pad)
                    nc.vector.tensor_copy(
                        out=mean_n[:, s:e], in_=mean_slab[:, s:e]
                    )
            else:
                mean_n = mean_slab

            for blk in range(NS_pad // 512):
                cols = slice(blk * 512, (blk + 1) * 512)
                o1h = []
                for h in range(2):
                    hs = slice(h * 128, (h + 1) * 128)
                    po1 = nps.tile([128, 512], f32, tag="po1")
                    nc.tensor.matmul(
                        out=po1[:],
                        lhsT=mmcast(w3m_sb[:, hs]),
                        rhs=mmcast(mean_n[:, cols]),
                        start=True,
                        stop=False,
                    )
                    nc.tensor.matmul(
                        out=po1[:],
                        lhsT=mmcast(w3x_sb[:, hs]),
                        rhs=mmcast(xT_sb[:, cols]),
                        start=False,
                        stop=True,
                    )
                    t = o1p.tile([128, 512], DT_N, tag=f"o1_{h}")
                    nc.scalar.activation(
                        out=t[:],
                        in_=po1[:],
                        func=mybir.ActivationFunctionType.Relu,
                        bias=b3_sb[:, h : h + 1],
                    )
                    o1h.append(t)
                for h in range(2):
                    hs = slice(h * 128, (h + 1) * 128)
                    po2 = nps.tile([128, 512], f32, tag="po2")
                    nc.tensor.matmul(
                        out=po2[:],
                        lhsT=mmcast(w4a_sb[:, hs]),
                        rhs=mmcast(o1h[0][:]),
                        start=True,
                        stop=False,
                    )
                    nc.tensor.matmul(
                        out=po2[:],
                        lhsT=mmcast(w4b_sb[:, hs]),
                        rhs=mmcast(o1h[1][:]),
                        start=False,
                        stop=True,
                    )
                    o2t = o2p.tile([128, 512], f32, tag="o2")
                    nc.scalar.activation(
                        out=o2t[:],
                        in_=po2[:],
                        func=mybir.ActivationFunctionType.Relu,
                        bias=b4_sb[:, h : h + 1],
                    )
                    nc.sync.dma_start(out=oT[hs, cols], in_=o2t[:])

    nc.finalize()
    return nc


# ---------------------------------------------------------------------------
# Entry point
# ---------------------------------------------------------------------------

def kernel(x, edge_index, edge_attr, W1, b1, W2, b2, W3, b3, W4, b4,
           edge_prec="bf16", node_prec="f32r"):
    x = np.asarray(x, dtype=np.float32)
    edge_index = np.asarray(edge_index)
    edge_attr = np.asarray(edge_attr, dtype=np.float32)
    W1 = np.asarray(W1, dtype=np.float32)
    b1 = np.asarray(b1, dtype=np.float32)
    W2 = np.asarray(W2, dtype=np.float32)
    b2 = np.asarray(b2, dtype=np.float32)
    W3 = np.asarray(W3, dtype=np.float32)
    b3 = np.asarray(b3, dtype=np.float32)
    W4 = np.asarray(W4, dtype=np.float32)
    b4 = np.asarray(b4, dtype=np.float32)

    row = np.asarray(edge_index[0], dtype=np.int64)
    col = np.asarray(edge_index[1], dtype=np.int64)

    edge_dt = ml_dtypes.bfloat16 if edge_prec == "bf16" else np.float32
    node_dt = ml_dtypes.bfloat16 if node_prec == "bf16" else np.float32
    f32r = node_prec == "f32r" or edge_prec == "f32r"

    in_maps, meta = _preprocess(x, row, col, edge_attr, edge_dt, node_dt)

    # weights: shared across cores
    w1c = np.vstack([W1[X_DIM:], W1[:X_DIM]]).astype(edge_dt)  # [43, 64]
    w2a = np.vstack([W2, b2[None, :]]).astype(edge_dt)  # [65, 128]
    w3m = W3[X_DIM:].astype(node_dt)
    w3x = W3[:X_DIM].astype(node_dt)
    w4a = W4[:128].astype(node_dt)
    w4b = W4[128:].astype(node_dt)
    b1d = b1.reshape(H1, 1).astype(np.float32)
    b3d = b3.reshape(2, 128).T.copy().astype(np.float32)
    b4d = b4.reshape(2, 128).T.copy().astype(np.float32)
    for m in in_maps:
        m.update(
            w1c=w1c, w2a=w2a, w3m=w3m, w3x=w3x, w4a=w4a, w4b=w4b,
            b1d=b1d, b3d=b3d, b4d=b4d,
        )

    nc = _build_program(
        meta, W1, b1, W2, b2, W3, b3, W4, b4,
        edge_dt, node_dt, matmul_f32r=f32r,
    )

    from concourse.bass_utils import run_bass_kernel_spmd

    res = run_bass_kernel_spmd(nc, in_maps, list(range(N_CORES)), **RUN_KWARGS)
    global LAST_EXEC_NS, LAST_RESULT
    LAST_EXEC_NS = res.exec_time_ns
    LAST_RESULT = res

    out = np.zeros((N_NODES, H3), dtype=np.float32)
    for c in range(N_CORES):
        oT_c = np.asarray(res.results[c]["oT"])  # [256, NS_pad]
        slots = meta["slot_tables"][c]
        valid = slots >= 0
        out[slots[valid]] = oT_c[:, valid].T
    return out


if __name__ == "__main__":
    # tiny self-test with a small synthetic graph via monkeypatched sizes
    pass


# revision 24
# speedup vs baseline: 1.0141x; 1.0141x over previous
"""GNN message-passing kernel for 8 Trainium2 NeuronCores.

Strategy (destination-sharded, degree-sorted):
  - Nodes are assigned to cores round-robin by degree rank, so every core's
    per-degree node counts match (after tiny padding) and one SPMD program
    serves all 8 cores.
  - Each core's edges are laid out grouped by destination node, nodes grouped
    by exact degree d.  The segment-sum over edges then becomes a dense
    [128ch, n_nodes, d] free-axis reduction - no scatter, no one-hot.
  - Edge MLP runs channels-on-partitions: in^T [43, E] tiles, two stationary
    weight matmuls (43->64, 65->128 with bias folded via a ones row).
  - Node MLP runs on the mean slab [128ch, node_slots] directly.
  - Host does index preprocessing, the x[col] gather into the edge-feature
    shard, and the final unpermute.
"""

import math
import os

import numpy as np
import ml_dtypes

N_NODES = 100000
N_EDGES = 1600000
N_CORES = 8
X_DIM, E_DIM = 4, 39
EAX_DIM = X_DIM + E_DIM  # 43
H1, H2 = 64, 128
H3 = 256

ROUND_EDGES = 1024  # compute-round edge budget (two PSUM banks at fp32)
DMA_EDGES = 4096    # edge-feature DMA granularity
RELU2_DVE_FRAC = 0.53  # fraction of second-relu columns evacuated on DVE

RUN_KWARGS: dict = {}
LAST_EXEC_NS = None
LAST_RESULT = None

F32 = "float32"


def _apply_tile_drain_patch():
    """walrus in this env only accepts one sync wait on a TPB_CTRL drain;
    split the Tile tail drain's waits across multiple drain instructions."""
    import bass_rust
    from concourse.tile import TileContext, ScopedClock

    if getattr(TileContext, "_drain_patch_applied", False):
        return

    def _patched(self, tick_clock, wait_clock):
        nc = self.nc
        drain_inst = nc.sync.drain()
        wait_clock.add_sem_waits(
            drain_inst.ins, ScopedClock({None: tick_clock.global_clock})
        )
        si = drain_inst.ins.sync_info
        waits = list(si.on_wait) if si is not None else []
        if len(waits) > 1:
            drain_inst.ins.sync_info = bass_rust.SyncInfo(
                on_wait=[waits[0]], on_update=[]
            )
            for w in waits[1:]:
                d2 = nc.sync.drain()
                d2.ins.sync_info = bass_rust.SyncInfo(on_wait=[w], on_update=[])
        nc.all_engine_barrier()
        assert self.sems is not None
        popped = nc._tile_sem_poison_stack.pop()
        assert popped is self._sem_poison
        nc.clear_and_free_semaphores(list(self.sems.allocated().values()))
        nc.all_engine_barrier()

    TileContext._drain_and_barrier = _patched
    TileContext._drain_patch_applied = True


# ---------------------------------------------------------------------------
# Host-side preprocessing
# ---------------------------------------------------------------------------

def _preprocess(x, row, col, edge_attr, edge_dt, node_dt):
    """Build per-core shards. Returns (in_maps, meta)."""
    deg = np.bincount(row, minlength=N_NODES).astype(np.int64)

    # Degree-ascending node order; node i of the order goes to core i % 8.
    node_order = np.argsort(deg, kind="stable")
    deg_sorted = deg[node_order]

    # Per-degree uniform region sizes M_d = max over cores of per-core count.
    degrees = np.unique(deg_sorted)
    # count of nodes with degree d on core c: split counts of each degree run
    # over cores: run of length L starting at global index s -> core (s+k)%8.
    run_starts = np.searchsorted(deg_sorted, degrees, side="left")
    run_lens = np.searchsorted(deg_sorted, degrees, side="right") - run_starts
    # M_d: ceil division accounting for phase; max over cores is
    # ceil(L/8) when L%8 != 0 aligned anywhere -> just use ceil(L/8) if the
    # run is spread evenly, but phase can make one core get one extra:
    # max count = ceil((L + (s % 8 accounted)) ... simply compute exactly.
    M = np.empty(len(degrees), dtype=np.int64)
    m_dc = np.empty((len(degrees), N_CORES), dtype=np.int64)
    for j, (s, L) in enumerate(zip(run_starts, run_lens)):
        idx = (s + np.arange(L)) % N_CORES
        cnt = np.bincount(idx, minlength=N_CORES)
        m_dc[j] = cnt
        M[j] = cnt.max()

    node_off = np.concatenate([[0], np.cumsum(M)])  # region node-slot offsets
    NS = int(node_off[-1])  # node slots per core (incl. per-degree pads)
    NS_pad = ((NS + 511) // 512) * 512
    edge_off = np.concatenate([[0], np.cumsum(M * degrees)])
    E_TOT = int(edge_off[-1])
    E_TOT_pad = ((E_TOT + DMA_EDGES - 1) // DMA_EDGES) * DMA_EDGES

    # --- per-core slot assignment -----------------------------------------
    # nodes of core c in degree order: node_order[c::8] with degree run
    # boundaries; slot of k-th node of degree d on core c = node_off[j] + k.
    in_maps = []
    slot_tables = []  # per core: global node id per slot (-1 pad)
    x = x.astype(np.float32)
    edge_attr = edge_attr.astype(np.float32)
    recip = 1.0 / np.maximum(deg, 1.0)

    # edge -> (core, slot) of its destination
    # global: position of node in sorted order
    pos_of_node = np.empty(N_NODES, dtype=np.int64)
    pos_of_node[node_order] = np.arange(N_NODES)
    core_of_node = pos_of_node % N_CORES

    # per-core, per-degree start index within the core's degree-sorted list
    for c in range(N_CORES):
        nodes_c = node_order[c::N_CORES]  # ascending degree
        deg_c = deg[nodes_c]
        # within-degree rank for this core's nodes
        # nodes_c sorted by degree; run starts:
        starts_c = np.searchsorted(deg_c, degrees, side="left")
        j_of_deg = np.searchsorted(degrees, deg_c)  # region index per node
        rank = np.arange(len(nodes_c)) - starts_c[j_of_deg]
        slot = node_off[j_of_deg] + rank

        slots = np.full(NS_pad, -1, dtype=np.int64)
        slots[slot] = nodes_c
        slot_tables.append(slots)

        # --- edges of this core ------------------------------------------
        emask = core_of_node[row] == c
        er = row[emask]
        ec = col[emask]
        ea = edge_attr[emask]
        # slot of dest node
        slot_of_node = np.full(N_NODES, -1, dtype=np.int64)
        slot_of_node[nodes_c] = slot
        es = slot_of_node[er]
        order = np.argsort(es, kind="stable")
        es_s = es[order]
        # within-node running index
        uniq, first_idx, counts = np.unique(
            es_s, return_index=True, return_counts=True
        )
        within = np.arange(len(es_s)) - np.repeat(first_idx, counts)
        # edge slot base per node slot: edge_off[j] + (slot-node_off[j])*d
        j_of_slot = j_of_deg[np.argsort(slot, kind="stable")]  # slot-> region
        # simpler: recompute region of each sorted edge's dest slot
        j_e = np.searchsorted(node_off[1:], es_s, side="right")
        d_e = degrees[j_e]
        ebase = edge_off[j_e] + (es_s - node_off[j_e]) * d_e
        epos = ebase + within

        eax = np.zeros((EAX_DIM, E_TOT_pad), dtype=np.float32)
        eax[:E_DIM, epos] = ea[order].T
        eax[E_DIM:, epos] = x[ec[order]].T

        xT = np.zeros((X_DIM, NS_pad), dtype=np.float32)
        valid = slots >= 0
        xT[:, valid] = x[slots[valid]].T
        rr = np.ones(NS_pad, dtype=np.float32)
        rr[valid] = recip[slots[valid]]
        rrep = np.broadcast_to(rr[None, :], (128, NS_pad)).copy()

        in_maps.append(
            {
                "eax": eax.astype(edge_dt),
                "xT": xT.astype(node_dt),
                "rrep": rrep.astype(edge_dt),
            }
        )

    meta = {
        "degrees": degrees.tolist(),
        "M": M.tolist(),
        "node_off": node_off.tolist(),
        "edge_off": edge_off.tolist(),
        "NS": NS,
        "NS_pad": NS_pad,
        "E_TOT": E_TOT,
        "E_TOT_pad": E_TOT_pad,
        "slot_tables": slot_tables,
    }
    return in_maps, meta


# ---------------------------------------------------------------------------
# Device program
# ---------------------------------------------------------------------------

def _build_program(meta, W1, b1, W2, b2, W3, b3, W4, b4, edge_dt, node_dt,
                   matmul_f32r=True):
    import concourse.bass as bass
    import concourse.mybir as mybir
    from concourse import bacc
    from concourse.tile import TileContext

    _apply_tile_drain_patch()

    DT_E = mybir.dt.bfloat16 if edge_dt == ml_dtypes.bfloat16 else mybir.dt.float32
    DT_N = mybir.dt.bfloat16 if node_dt == ml_dtypes.bfloat16 else mybir.dt.float32
    f32 = mybir.dt.float32

    NS_pad = meta["NS_pad"]
    E_TOT_pad = meta["E_TOT_pad"]
    degrees = meta["degrees"]
    M = meta["M"]
    node_off = meta["node_off"]
    edge_off = meta["edge_off"]

    nc = bacc.Bacc("TRN2", target_bir_lowering=False, debug=False)
    eax = nc.declare_dram_parameter("eax", [EAX_DIM, E_TOT_pad], DT_E, isOutput=False)
    xT = nc.declare_dram_parameter("xT", [X_DIM, NS_pad], DT_N, isOutput=False)
    rrep = nc.declare_dram_parameter("rrep", [128, NS_pad], DT_E, isOutput=False)
    w1c = nc.declare_dram_parameter("w1c", [EAX_DIM, H1], DT_E, isOutput=False)
    b1d = nc.declare_dram_parameter("b1d", [128, 1], f32, isOutput=False)
    w2d = nc.declare_dram_parameter("w2d", [2 * H1, H2], DT_E, isOutput=False)
    b2d = nc.declare_dram_parameter("b2d", [H2, 1], f32, isOutput=False)
    w3m = nc.declare_dram_parameter("w3m", [H2, H3], DT_N, isOutput=False)
    w3x = nc.declare_dram_parameter("w3x", [X_DIM, H3], DT_N, isOutput=False)
    w4a = nc.declare_dram_parameter("w4a", [128, H3], DT_N, isOutput=False)
    w4b = nc.declare_dram_parameter("w4b", [128, H3], DT_N, isOutput=False)
    b3d = nc.declare_dram_parameter("b3d", [128, 2], f32, isOutput=False)
    b4d = nc.declare_dram_parameter("b4d", [128, 2], f32, isOutput=False)
    oT = nc.declare_dram_parameter("oT", [H3, NS_pad], f32, isOutput=True)

    def mmcast(ap):
        if matmul_f32r and ap.dtype == mybir.dt.float32:
            return ap.bitcast(mybir.dt.float32r)
        return ap

    # build edge-round schedule: list of (dma ranges) and rounds
    # round: (edge_start, n_nodes, d, node_slot_start) all within one region
    rounds = []
    for j, d in enumerate(degrees):
        if d == 0:
            continue
        m = M[j]
        npr = max(1, ROUND_EDGES // d)  # nodes per round
        s = 0
        while s < m:
            n = min(npr, m - s)
            rounds.append((edge_off[j] + s * d, n, d, node_off[j] + s))
            s += n
    # pack rounds into DMA chunks of <= DMA_EDGES contiguous edges
    chunks = []  # (dma_start, dma_len, [round indices])
    cur = None
    for ri, (e0, n, d, s0) in enumerate(rounds):
        ln = n * d
        if cur is not None and e0 + ln - cur[0] <= DMA_EDGES:
            cur[1] = e0 + ln - cur[0]
            cur[2].append(ri)
        else:
            if cur is not None:
                chunks.append(tuple(cur))
            cur = [e0, ln, [ri]]
    if cur is not None:
        chunks.append(tuple(cur))

    with TileContext(nc) as tc:
      with tc.tile_pool(name="slab", bufs=1) as slabp:
        mean_slab = slabp.tile([128, NS_pad], DT_E)
        with (
            tc.tile_pool(name="const", bufs=1) as constp,
            tc.tile_pool(name="eaxp", bufs=3) as eaxp,
            tc.tile_pool(name="h1p", bufs=3) as h1p,
            tc.tile_pool(name="h2p", bufs=3) as h2p,
            tc.tile_pool(name="ph1p", bufs=2, space="PSUM") as ph1p,
            tc.tile_pool(name="ph2p", bufs=2, space="PSUM") as ph2p,
        ):
            w1c_sb = constp.tile([EAX_DIM, H1], DT_E)
            nc.gpsimd.dma_start(out=w1c_sb[:], in_=w1c[:])
            w2_sb = constp.tile([2 * H1, H2], DT_E)
            nc.gpsimd.dma_start(out=w2_sb[:], in_=w2d[:])
            b1_sb = constp.tile([128, 1], f32)  # b1 stacked twice (pairing)
            nc.gpsimd.dma_start(out=b1_sb[:], in_=b1d[:])
            b2_sb = constp.tile([H2, 1], f32)
            nc.gpsimd.dma_start(out=b2_sb[:], in_=b2d[:])

            nc.gpsimd.memset(mean_slab[:], 0.0)

            def do_relu2_and_reduce(ph2t, re, n, d, s0):
                h2t = h2p.tile([H2, ROUND_EDGES], DT_E, tag="h2")
                ca = re - int(re * RELU2_DVE_FRAC)
                if ca > 0:
                    nc.scalar.activation(
                        out=h2t[:, :ca],
                        in_=ph2t[:, :ca],
                        func=mybir.ActivationFunctionType.Relu,
                        bias=b2_sb[:],
                    )
                if re - ca > 0:
                    nc.vector.tensor_scalar(
                        out=h2t[:, ca:re],
                        in0=ph2t[:, ca:re],
                        scalar1=b2_sb[:],
                        scalar2=0.0,
                        op0=mybir.AluOpType.add,
                        op1=mybir.AluOpType.max,
                    )
                src = h2t[:, :re].rearrange("p (n d) -> p n d", d=d)
                with nc.allow_low_precision(reason="bf16 mean slab"):
                    nc.vector.tensor_reduce(
                        out=mean_slab[:, s0 : s0 + n],
                        in_=src,
                        op=mybir.AluOpType.add,
                        axis=mybir.AxisListType.X,
                    )

            def mm_split(out_ap, lhsT, rhs_tile, ro, re, pbase):
                # matmul in <=512-column pieces (one PSUM bank each)
                for j in range(0, re, 512):
                    w = min(512, re - j)
                    nc.tensor.matmul(
                        out=out_ap[pbase : pbase + lhsT.shape[1], j : j + w],
                        lhsT=mmcast(lhsT),
                        rhs=mmcast(rhs_tile[:, ro + j : ro + j + w]),
                        start=True,
                        stop=True,
                    )

            for (c0, clen, ris) in chunks:
                eax_t = eaxp.tile([EAX_DIM, DMA_EDGES], DT_E, tag="eax")
                nc.sync.dma_start(out=eax_t[:, :clen], in_=eax[:, c0 : c0 + clen])
                for k in range(0, len(ris), 2):
                    pair = ris[k : k + 2]
                    rs = [rounds[ri] for ri in pair]
                    res = [n * d for (_, n, d, _) in rs]
                    remax = max(res)
                    ph1 = ph1p.tile([128, ROUND_EDGES], f32, tag="ph1")
                    for half, ((e0, n, d, s0), re) in enumerate(zip(rs, res)):
                        mm_split(ph1[:], w1c_sb[:], eax_t, e0 - c0, re, half * H1)
                    h1t = h1p.tile([128, ROUND_EDGES], DT_E, tag="h1")
                    nparts = 128 if len(rs) == 2 else H1
                    nc.scalar.activation(
                        out=h1t[:nparts, :remax],
                        in_=ph1[:nparts, :remax],
                        func=mybir.ActivationFunctionType.Relu,
                        bias=b1_sb[:nparts],
                    )
                    for half, ((e0, n, d, s0), re) in enumerate(zip(rs, res)):
                        ph2 = ph2p.tile([H2, ROUND_EDGES], f32, tag="ph2")
                        for j in range(0, re, 512):
                            w = min(512, re - j)
                            nc.tensor.matmul(
                                out=ph2[:, j : j + w],
                                lhsT=mmcast(w2_sb[half * H1 : (half + 1) * H1, :]),
                                rhs=mmcast(
                                    h1t[half * H1 : half * H1 + H1, j : j + w]
                                ),
                                start=True,
                                stop=True,
                            )
                        if d > ROUND_EDGES:
                            raise NotImplementedError("degree > ROUND_EDGES")
                        do_relu2_and_reduce(ph2[:], re, n, d, s0)

            # mean scaling by 1/deg
            rrep_sb = constp.tile([128, NS_pad], DT_E)
            nc.gpsimd.dma_start(out=rrep_sb[:], in_=rrep[:])
            with nc.allow_low_precision(reason="bf16 mean slab"):
                for s in range(0, NS_pad, 2048):
                    e = min(s + 2048, NS_pad)
                    nc.vector.tensor_tensor(
                        out=mean_slab[:, s:e],
                        in0=mean_slab[:, s:e],
                        in1=rrep_sb[:, s:e],
                        op=mybir.AluOpType.mult,
                    )

        # ---- node MLP phase ----
        with (
            tc.tile_pool(name="nconst", bufs=1) as nconstp,
            tc.tile_pool(name="o1p", bufs=4) as o1p,
            tc.tile_pool(name="o2p", bufs=3) as o2p,
            tc.tile_pool(name="nps", bufs=4, space="PSUM") as nps,
        ):
            w3m_sb = nconstp.tile([H2, H3], DT_N)
            nc.gpsimd.dma_start(out=w3m_sb[:], in_=w3m[:])
            w3x_sb = nconstp.tile([X_DIM, H3], DT_N)
            nc.gpsimd.dma_start(out=w3x_sb[:], in_=w3x[:])
            w4a_sb = nconstp.tile([128, H3], DT_N)
            nc.gpsimd.dma_start(out=w4a_sb[:], in_=w4a[:])
            w4b_sb = nconstp.tile([128, H3], DT_N)
            nc.gpsimd.dma_start(out=w4b_sb[:], in_=w4b[:])
            b3_sb = nconstp.tile([128, 2], f32)
            nc.gpsimd.dma_start(out=b3_sb[:], in_=b3d[:])
            b4_sb = nconstp.tile([128, 2], f32)
            nc.gpsimd.dma_start(out=b4_sb[:], in_=b4d[:])
            xT_sb = nconstp.tile([X_DIM, NS_pad], DT_N)
            nc.gpsimd.dma_start(out=xT_sb[:], in_=xT[:])

            # mean slab may need dtype cast for node matmuls
            if DT_N != DT_E:
                mean_n = nconstp.tile([128, NS_pad], DT_N)
                for s in range(0, NS_pad, 2048):
                    e = min(s + 2048, NS_pad)
                    nc.vector.tensor_copy(
                        out=mean_n[:, s:e], in_=mean_slab[:, s:e]
                    )
            else:
                mean_n = mean_slab

            for blk in range(NS_pad // 512):
                cols = slice(blk * 512, (blk + 1) * 512)
                o1h = []
                for h in range(2):
                    hs = slice(h * 128, (h + 1) * 128)
                    po1 = nps.tile([128, 512], f32, tag="po1")
                    nc.tensor.matmul(
                        out=po1[:],
                        lhsT=mmcast(w3m_sb[:, hs]),
                        rhs=mmcast(mean_n[:, cols]),
                        start=True,
                        stop=False,
                    )
                    nc.tensor.matmul(
                        out=po1[:],
                        lhsT=mmcast(w3x_sb[:, hs]),
                        rhs=mmcast(xT_sb[:, cols]),
                        start=False,
                        stop=True,
                    )
                    t = o1p.tile([128, 512], DT_N, tag=f"o1_{h}")
                    nc.vector.tensor_scalar(
                        out=t[:],
                        in0=po1[:],
                        scalar1=b3_sb[:, h : h + 1],
                        scalar2=0.0,
                        op0=mybir.AluOpType.add,
                        op1=mybir.AluOpType.max,
                    )
                    o1h.append(t)
                for h in range(2):
                    hs = slice(h * 128, (h + 1) * 128)
                    po2 = nps.tile([128, 512], f32, tag="po2")
                    nc.tensor.matmul(
                        out=po2[:],
                        lhsT=mmcast(w4a_sb[:, hs]),
                        rhs=mmcast(o1h[0][:]),
                        start=True,
                        stop=False,
                    )
                    nc.tensor.matmul(
                        out=po2[:],
                        lhsT=mmcast(w4b_sb[:, hs]),
                        rhs=mmcast(o1h[1][:]),
                        start=False,
                        stop=True,
                    )
                    o2t = o2p.tile([128, 512], f32, tag="o2")
                    nc.scalar.activation(
                        out=o2t[:],
                        in_=po2[:],
                        func=mybir.ActivationFunctionType.Relu,
                        bias=b4_sb[:, h : h + 1],
                    )
                    nc.sync.dma_start(out=oT[hs, cols], in_=o2t[:])

    nc.finalize()
    return nc


# ---------------------------------------------------------------------------
# Entry point
# ---------------------------------------------------------------------------

def kernel(x, edge_index, edge_attr, W1, b1, W2, b2, W3, b3, W4, b4,
           edge_prec="bf16", node_prec="f32r"):
    x = np.asarray(x, dtype=np.float32)
    edge_index = np.asarray(edge_index)
    edge_attr = np.asarray(edge_attr, dtype=np.float32)
    W1 = np.asarray(W1, dtype=np.float32)
    b1 = np.asarray(b1, dtype=np.float32)
    W2 = np.asarray(W2, dtype=np.float32)
    b2 = np.asarray(b2, dtype=np.float32)
    W3 = np.asarray(W3, dtype=np.float32)
    b3 = np.asarray(b3, dtype=np.float32)
    W4 = np.asarray(W4, dtype=np.float32)
    b4 = np.asarray(b4, dtype=np.float32)

    row = np.asarray(edge_index[0], dtype=np.int64)
    col = np.asarray(edge_index[1], dtype=np.int64)

    edge_dt = ml_dtypes.bfloat16 if edge_prec == "bf16" else np.float32
    node_dt = ml_dtypes.bfloat16 if node_prec == "bf16" else np.float32
    f32r = node_prec == "f32r" or edge_prec == "f32r"

    in_maps, meta = _preprocess(x, row, col, edge_attr, edge_dt, node_dt)

    # weights: shared across cores
    w1c = np.vstack([W1[X_DIM:], W1[:X_DIM]]).astype(edge_dt)  # [43, 64]
    w2d = np.vstack([W2, W2]).astype(edge_dt)  # [128, 128], one copy per pair half
    w3m = W3[X_DIM:].astype(node_dt)
    w3x = W3[:X_DIM].astype(node_dt)
    w4a = W4[:128].astype(node_dt)
    w4b = W4[128:].astype(node_dt)
    b1d = np.concatenate([b1, b1]).reshape(128, 1).astype(np.float32)
    b2d = b2.reshape(H2, 1).astype(np.float32)
    b3d = b3.reshape(2, 128).T.copy().astype(np.float32)
    b4d = b4.reshape(2, 128).T.copy().astype(np.float32)
    for m in in_maps:
        m.update(
            w1c=w1c, w2d=w2d, w3m=w3m, w3x=w3x, w4a=w4a, w4b=w4b,
            b1d=b1d, b2d=b2d, b3d=b3d, b4d=b4d,
        )

    nc = _build_program(
        meta, W1, b1, W2, b2, W3, b3, W4, b4,
        edge_dt, node_dt, matmul_f32r=f32r,
    )

    from concourse.bass_utils import run_bass_kernel_spmd

    res = run_bass_kernel_spmd(nc, in_maps, list(range(N_CORES)), **RUN_KWARGS)
    global LAST_EXEC_NS, LAST_RESULT
    LAST_EXEC_NS = res.exec_time_ns
    LAST_RESULT = res

    out = np.zeros((N_NODES, H3), dtype=np.float32)
    for c in range(N_CORES):
        oT_c = np.asarray(res.results[c]["oT"])  # [256, NS_pad]
        slots = meta["slot_tables"][c]
        valid = slots >= 0
        out[slots[valid]] = oT_c[:, valid].T
    return out


if __name__ == "__main__":
    # tiny self-test with a small synthetic graph via monkeypatched sizes
    pass


# revision 33
# speedup vs baseline: 1.5874x; 1.5653x over previous
"""GNN message-passing kernel for 8 Trainium2 NeuronCores.

Strategy (destination-sharded, degree-sorted):
  - Nodes are assigned to cores round-robin by degree rank, so every core's
    per-degree node counts match (after tiny padding) and one SPMD program
    serves all 8 cores.
  - Each core's edges are laid out grouped by destination node, nodes grouped
    by exact degree d.  The segment-sum over edges then becomes a dense
    [128ch, n_nodes, d] free-axis reduction - no scatter, no one-hot.
  - Edge MLP runs channels-on-partitions: in^T [43, E] tiles, two stationary
    weight matmuls (43->64, 65->128 with bias folded via a ones row).
  - Node MLP runs on the mean slab [128ch, node_slots] directly.
  - Host does index preprocessing, the x[col] gather into the edge-feature
    shard, and the final unpermute.
"""

import math
import os

import numpy as np
import ml_dtypes

N_NODES = 100000
N_EDGES = 1600000
N_CORES = 8
X_DIM, E_DIM = 4, 39
EAX_DIM = X_DIM + E_DIM  # 43
H1, H2 = 64, 128
H3 = 256

ROUND_EDGES = 1024  # compute-round edge budget (two PSUM banks at fp32)
DMA_EDGES = 4096    # edge-feature DMA granularity
RELU2_DVE_FRAC = 0.53  # fraction of second-relu columns evacuated on DVE

RUN_KWARGS: dict = {}
LAST_EXEC_NS = None
LAST_RESULT = None

F32 = "float32"


def _apply_tile_drain_patch():
    """walrus in this env only accepts one sync wait on a TPB_CTRL drain;
    split the Tile tail drain's waits across multiple drain instructions."""
    import bass_rust
    from concourse.tile import TileContext, ScopedClock

    if getattr(TileContext, "_drain_patch_applied", False):
        return

    def _patched(self, tick_clock, wait_clock):
        nc = self.nc
        drain_inst = nc.sync.drain()
        wait_clock.add_sem_waits(
            drain_inst.ins, ScopedClock({None: tick_clock.global_clock})
        )
        si = drain_inst.ins.sync_info
        waits = list(si.on_wait) if si is not None else []
        if len(waits) > 1:
            drain_inst.ins.sync_info = bass_rust.SyncInfo(
                on_wait=[waits[0]], on_update=[]
            )
            for w in waits[1:]:
                d2 = nc.sync.drain()
                d2.ins.sync_info = bass_rust.SyncInfo(on_wait=[w], on_update=[])
        nc.all_engine_barrier()
        assert self.sems is not None
        popped = nc._tile_sem_poison_stack.pop()
        assert popped is self._sem_poison
        nc.clear_and_free_semaphores(list(self.sems.allocated().values()))
        nc.all_engine_barrier()

    TileContext._drain_and_barrier = _patched
    TileContext._drain_patch_applied = True


# ---------------------------------------------------------------------------
# Host-side preprocessing
# ---------------------------------------------------------------------------

def _preprocess(x, row, col, edge_attr, edge_dt, node_dt):
    """Build per-core shards. Returns (in_maps, meta)."""
    deg = np.bincount(row, minlength=N_NODES).astype(np.int64)

    # Degree-ascending node order; node i of the order goes to core i % 8.
    node_order = np.argsort(deg, kind="stable")
    deg_sorted = deg[node_order]

    # Per-degree uniform region sizes M_d = max over cores of per-core count.
    degrees = np.unique(deg_sorted)
    # count of nodes with degree d on core c: split counts of each degree run
    # over cores: run of length L starting at global index s -> core (s+k)%8.
    run_starts = np.searchsorted(deg_sorted, degrees, side="left")
    run_lens = np.searchsorted(deg_sorted, degrees, side="right") - run_starts
    # M_d: ceil division accounting for phase; max over cores is
    # ceil(L/8) when L%8 != 0 aligned anywhere -> just use ceil(L/8) if the
    # run is spread evenly, but phase can make one core get one extra:
    # max count = ceil((L + (s % 8 accounted)) ... simply compute exactly.
    M = np.empty(len(degrees), dtype=np.int64)
    m_dc = np.empty((len(degrees), N_CORES), dtype=np.int64)
    for j, (s, L) in enumerate(zip(run_starts, run_lens)):
        idx = (s + np.arange(L)) % N_CORES
        cnt = np.bincount(idx, minlength=N_CORES)
        m_dc[j] = cnt
        M[j] = cnt.max()

    node_off = np.concatenate([[0], np.cumsum(M)])  # region node-slot offsets
    NS = int(node_off[-1])  # node slots per core (incl. per-degree pads)
    NS_pad = ((NS + 511) // 512) * 512
    edge_off = np.concatenate([[0], np.cumsum(M * degrees)])
    E_TOT = int(edge_off[-1])
    E_TOT_pad = ((E_TOT + DMA_EDGES - 1) // DMA_EDGES) * DMA_EDGES

    # --- compute rounds and DMA chunk packing (shared with the builder) ---
    # round: (edge_start, n_nodes, d, node_slot_start)
    rounds = []
    for j, d in enumerate(degrees):
        if d == 0:
            continue
        m = int(M[j])
        npr = max(1, ROUND_EDGES // int(d))
        s = 0
        while s < m:
            n = min(npr, m - s)
            rounds.append((int(edge_off[j] + s * d), n, int(d), int(node_off[j] + s)))
            s += n
    # chunks of whole rounds, <= DMA_EDGES contiguous edges
    chunks = []  # [c0, clen, [round indices]]
    cur = None
    for ri, (e0, n, d, s0) in enumerate(rounds):
        ln = n * d
        if cur is not None and e0 + ln - cur[0] <= DMA_EDGES:
            cur[1] = e0 + ln - cur[0]
            cur[2].append(ri)
        else:
            if cur is not None:
                chunks.append(cur)
            cur = [e0, ln, [ri]]
    if cur is not None:
        chunks.append(cur)
    # pack chunk pairs into 128-partition column blocks: chunk 2s -> rows
    # [0:43], chunk 2s+1 -> rows [64:107], both at col base scol[s]
    scols = []
    rowbases = []
    base = 0
    for ci, (c0, clen, ris) in enumerate(chunks):
        if ci % 2 == 0:
            scols.append(base)
            rowbases.append(0)
            width = clen
        else:
            scols.append(scols[-1])
            rowbases.append(64)
            width = max(width, clen)
        if ci % 2 == 1 or ci == len(chunks) - 1:
            base += width
    EC = ((base + DMA_EDGES - 1) // DMA_EDGES) * DMA_EDGES  # eax column count

    # --- per-core slot assignment -----------------------------------------
    # nodes of core c in degree order: node_order[c::8] with degree run
    # boundaries; slot of k-th node of degree d on core c = node_off[j] + k.
    in_maps = []
    slot_tables = []  # per core: global node id per slot (-1 pad)
    x = x.astype(np.float32)
    edge_attr = edge_attr.astype(np.float32)
    recip = 1.0 / np.maximum(deg, 1.0)

    # edge -> (core, slot) of its destination
    # global: position of node in sorted order
    pos_of_node = np.empty(N_NODES, dtype=np.int64)
    pos_of_node[node_order] = np.arange(N_NODES)
    core_of_node = pos_of_node % N_CORES

    # per-core, per-degree start index within the core's degree-sorted list
    for c in range(N_CORES):
        nodes_c = node_order[c::N_CORES]  # ascending degree
        deg_c = deg[nodes_c]
        # within-degree rank for this core's nodes
        # nodes_c sorted by degree; run starts:
        starts_c = np.searchsorted(deg_c, degrees, side="left")
        j_of_deg = np.searchsorted(degrees, deg_c)  # region index per node
        rank = np.arange(len(nodes_c)) - starts_c[j_of_deg]
        slot = node_off[j_of_deg] + rank

        slots = np.full(NS_pad, -1, dtype=np.int64)
        slots[slot] = nodes_c
        slot_tables.append(slots)

        # --- edges of this core ------------------------------------------
        emask = core_of_node[row] == c
        er = row[emask]
        ec = col[emask]
        ea = edge_attr[emask]
        # slot of dest node
        slot_of_node = np.full(N_NODES, -1, dtype=np.int64)
        slot_of_node[nodes_c] = slot
        es = slot_of_node[er]
        order = np.argsort(es, kind="stable")
        es_s = es[order]
        # within-node running index
        uniq, first_idx, counts = np.unique(
            es_s, return_index=True, return_counts=True
        )
        within = np.arange(len(es_s)) - np.repeat(first_idx, counts)
        # edge slot base per node slot: edge_off[j] + (slot-node_off[j])*d
        j_of_slot = j_of_deg[np.argsort(slot, kind="stable")]  # slot-> region
        # simpler: recompute region of each sorted edge's dest slot
        j_e = np.searchsorted(node_off[1:], es_s, side="right")
        d_e = degrees[j_e]
        ebase = edge_off[j_e] + (es_s - node_off[j_e]) * d_e
        epos = ebase + within

        chunk_starts = np.array([ch[0] for ch in chunks], dtype=np.int64)
        sc_arr = np.array(scols, dtype=np.int64)
        rb_arr = np.array(rowbases, dtype=np.int64)
        ci_e = np.searchsorted(chunk_starts, epos, side="right") - 1
        cols_e = sc_arr[ci_e] + (epos - chunk_starts[ci_e])
        rb_e = rb_arr[ci_e]
        ea_s = ea[order]
        xg_s = x[ec[order]]
        eax = np.zeros((128, EC), dtype=np.float32)
        for b in (0, 64):
            msk = rb_e == b
            eax[b : b + E_DIM, cols_e[msk]] = ea_s[msk].T
            eax[b + E_DIM : b + EAX_DIM, cols_e[msk]] = xg_s[msk].T

        xT = np.zeros((X_DIM, NS_pad), dtype=np.float32)
        valid = slots >= 0
        xT[:, valid] = x[slots[valid]].T
        rr = np.ones(NS_pad, dtype=np.float32)
        rr[valid] = recip[slots[valid]]
        rrep = np.broadcast_to(rr[None, :], (128, NS_pad)).copy()

        in_maps.append(
            {
                "eax": eax.astype(edge_dt),
                "xT": xT.astype(node_dt),
                "rrep": rrep.astype(edge_dt),
            }
        )

    meta = {
        "NS": NS,
        "NS_pad": NS_pad,
        "E_TOT": E_TOT,
        "EC": EC,
        "rounds": rounds,
        "chunks": chunks,
        "scols": scols,
        "rowbases": rowbases,
        "slot_tables": slot_tables,
    }
    return in_maps, meta


# ---------------------------------------------------------------------------
# Device program
# ---------------------------------------------------------------------------

def _build_program(meta, W1, b1, W2, b2, W3, b3, W4, b4, edge_dt, node_dt,
                   matmul_f32r=True):
    import concourse.bass as bass
    import concourse.mybir as mybir
    from concourse import bacc
    from concourse.tile import TileContext

    _apply_tile_drain_patch()

    DT_E = mybir.dt.bfloat16 if edge_dt == ml_dtypes.bfloat16 else mybir.dt.float32
    DT_N = mybir.dt.bfloat16 if node_dt == ml_dtypes.bfloat16 else mybir.dt.float32
    f32 = mybir.dt.float32

    NS_pad = meta["NS_pad"]
    EC = meta["EC"]
    rounds = meta["rounds"]
    chunks = meta["chunks"]
    scols = meta["scols"]
    rowbases = meta["rowbases"]

    nc = bacc.Bacc("TRN2", target_bir_lowering=False, debug=False)
    eax = nc.declare_dram_parameter("eax", [128, EC], DT_E, isOutput=False)
    xT = nc.declare_dram_parameter("xT", [X_DIM, NS_pad], DT_N, isOutput=False)
    rrep = nc.declare_dram_parameter("rrep", [128, NS_pad], DT_E, isOutput=False)
    w1c = nc.declare_dram_parameter("w1c", [128, H1], DT_E, isOutput=False)
    b1d = nc.declare_dram_parameter("b1d", [128, 1], f32, isOutput=False)
    w2d = nc.declare_dram_parameter("w2d", [2 * H1, H2], DT_E, isOutput=False)
    b2d = nc.declare_dram_parameter("b2d", [H2, 1], f32, isOutput=False)
    w3m = nc.declare_dram_parameter("w3m", [H2, H3], DT_N, isOutput=False)
    w3x = nc.declare_dram_parameter("w3x", [X_DIM, H3], DT_N, isOutput=False)
    w4a = nc.declare_dram_parameter("w4a", [128, H3], DT_N, isOutput=False)
    w4b = nc.declare_dram_parameter("w4b", [128, H3], DT_N, isOutput=False)
    b3d = nc.declare_dram_parameter("b3d", [128, 2], f32, isOutput=False)
    b4d = nc.declare_dram_parameter("b4d", [128, 2], f32, isOutput=False)
    oT = nc.declare_dram_parameter("oT", [H3, NS_pad], f32, isOutput=True)

    def mmcast(ap):
        if matmul_f32r and ap.dtype == mybir.dt.float32:
            return ap.bitcast(mybir.dt.float32r)
        return ap

    with TileContext(nc) as tc:
      with tc.tile_pool(name="slab", bufs=1) as slabp:
        mean_slab = slabp.tile([128, NS_pad], DT_E)
        with (
            tc.tile_pool(name="const", bufs=1) as constp,
            tc.tile_pool(name="eaxp", bufs=3) as eaxp,
            tc.tile_pool(name="h1p", bufs=3) as h1p,
            tc.tile_pool(name="h2p", bufs=3) as h2p,
            tc.tile_pool(name="ph1p", bufs=2, space="PSUM") as ph1p,
            tc.tile_pool(name="ph2p", bufs=2, space="PSUM") as ph2p,
        ):
            w1c_sb = constp.tile([128, H1], DT_E)
            nc.gpsimd.dma_start(out=w1c_sb[:], in_=w1c[:])
            w2_sb = constp.tile([2 * H1, H2], DT_E)
            nc.gpsimd.dma_start(out=w2_sb[:], in_=w2d[:])
            b1_sb = constp.tile([128, 1], f32)  # b1 stacked twice (pairing)
            nc.gpsimd.dma_start(out=b1_sb[:], in_=b1d[:])
            b2_sb = constp.tile([H2, 1], f32)
            nc.gpsimd.dma_start(out=b2_sb[:], in_=b2d[:])

            nc.gpsimd.memset(mean_slab[:], 0.0)

            def do_relu2_and_reduce(ph2t, re, n, d, s0):
                h2t = h2p.tile([H2, ROUND_EDGES], DT_E, tag="h2")
                ca = re - int(re * RELU2_DVE_FRAC)
                if ca > 0:
                    nc.scalar.activation(
                        out=h2t[:, :ca],
                        in_=ph2t[:, :ca],
                        func=mybir.ActivationFunctionType.Relu,
                        bias=b2_sb[:],
                    )
                if re - ca > 0:
                    nc.vector.tensor_scalar(
                        out=h2t[:, ca:re],
                        in0=ph2t[:, ca:re],
                        scalar1=b2_sb[:],
                        scalar2=0.0,
                        op0=mybir.AluOpType.add,
                        op1=mybir.AluOpType.max,
                    )
                src = h2t[:, :re].rearrange("p (n d) -> p n d", d=d)
                with nc.allow_low_precision(reason="bf16 mean slab"):
                    nc.vector.tensor_reduce(
                        out=mean_slab[:, s0 : s0 + n],
                        in_=src,
                        op=mybir.AluOpType.add,
                        axis=mybir.AxisListType.X,
                    )

            for ci, (c0, clen, ris) in enumerate(chunks):
                if rowbases[ci] == 0:
                    w = clen
                    if ci + 1 < len(chunks) and rowbases[ci + 1] == 64:
                        w = max(w, chunks[ci + 1][1])
                    eax_t = eaxp.tile([128, DMA_EDGES], DT_E, tag="eax")
                    nc.sync.dma_start(
                        out=eax_t[:, :w], in_=eax[:, scols[ci] : scols[ci] + w]
                    )
                rb = rowbases[ci]
                for k in range(0, len(ris), 2):
                    pair = ris[k : k + 2]
                    rs = [rounds[ri] for ri in pair]
                    res = [n * d for (_, n, d, _) in rs]
                    remax = max(res)
                    ph1 = ph1p.tile([128, ROUND_EDGES], f32, tag="ph1")
                    for half, ((e0, n, d, s0), re) in enumerate(zip(rs, res)):
                        for j in range(0, re, 512):
                            wj = min(512, re - j)
                            ro = e0 - c0
                            nc.tensor.matmul(
                                out=ph1[half * H1 : half * H1 + H1, j : j + wj],
                                lhsT=mmcast(w1c_sb[rb : rb + EAX_DIM, :]),
                                rhs=mmcast(eax_t[rb : rb + EAX_DIM, ro + j : ro + j + wj]),
                                start=True,
                                stop=True,
                            )
                    h1t = h1p.tile([128, ROUND_EDGES], DT_E, tag="h1")
                    nparts = 128 if len(rs) == 2 else H1
                    nc.scalar.activation(
                        out=h1t[:nparts, :remax],
                        in_=ph1[:nparts, :remax],
                        func=mybir.ActivationFunctionType.Relu,
                        bias=b1_sb[:nparts],
                    )
                    for half, ((e0, n, d, s0), re) in enumerate(zip(rs, res)):
                        ph2 = ph2p.tile([H2, ROUND_EDGES], f32, tag="ph2")
                        for j in range(0, re, 512):
                            w = min(512, re - j)
                            nc.tensor.matmul(
                                out=ph2[:, j : j + w],
                                lhsT=mmcast(w2_sb[half * H1 : (half + 1) * H1, :]),
                                rhs=mmcast(
                                    h1t[half * H1 : half * H1 + H1, j : j + w]
                                ),
                                start=True,
                                stop=True,
                            )
                        if d > ROUND_EDGES:
                            raise NotImplementedError("degree > ROUND_EDGES")
                        do_relu2_and_reduce(ph2[:], re, n, d, s0)

            # mean scaling by 1/deg
            rrep_sb = constp.tile([128, NS_pad], DT_E)
            nc.gpsimd.dma_start(out=rrep_sb[:], in_=rrep[:])
            with nc.allow_low_precision(reason="bf16 mean slab"):
                for s in range(0, NS_pad, 2048):
                    e = min(s + 2048, NS_pad)
                    nc.vector.tensor_tensor(
                        out=mean_slab[:, s:e],
                        in0=mean_slab[:, s:e],
                        in1=rrep_sb[:, s:e],
                        op=mybir.AluOpType.mult,
                    )

        # ---- node MLP phase ----
        with (
            tc.tile_pool(name="nconst", bufs=1) as nconstp,
            tc.tile_pool(name="o1p", bufs=4) as o1p,
            tc.tile_pool(name="o2p", bufs=3) as o2p,
            tc.tile_pool(name="nps", bufs=4, space="PSUM") as nps,
        ):
            w3m_sb = nconstp.tile([H2, H3], DT_N)
            nc.gpsimd.dma_start(out=w3m_sb[:], in_=w3m[:])
            w3x_sb = nconstp.tile([X_DIM, H3], DT_N)
            nc.gpsimd.dma_start(out=w3x_sb[:], in_=w3x[:])
            w4a_sb = nconstp.tile([128, H3], DT_N)
            nc.gpsimd.dma_start(out=w4a_sb[:], in_=w4a[:])
            w4b_sb = nconstp.tile([128, H3], DT_N)
            nc.gpsimd.dma_start(out=w4b_sb[:], in_=w4b[:])
            b3_sb = nconstp.tile([128, 2], f32)
            nc.gpsimd.dma_start(out=b3_sb[:], in_=b3d[:])
            b4_sb = nconstp.tile([128, 2], f32)
            nc.gpsimd.dma_start(out=b4_sb[:], in_=b4d[:])
            xT_sb = nconstp.tile([X_DIM, NS_pad], DT_N)
            nc.gpsimd.dma_start(out=xT_sb[:], in_=xT[:])

            # mean slab may need dtype cast for node matmuls
            if DT_N != DT_E:
                mean_n = nconstp.tile([128, NS_pad], DT_N)
                for s in range(0, NS_pad, 2048):
                    e = min(s + 2048, NS_pad)
                    nc.vector.tensor_copy(
                        out=mean_n[:, s:e], in_=mean_slab[:, s:e]
                    )
            else:
                mean_n = mean_slab

            for blk in range(NS_pad // 512):
                cols = slice(blk * 512, (blk + 1) * 512)
                o1h = []
                for h in range(2):
                    hs = slice(h * 128, (h + 1) * 128)
                    po1 = nps.tile([128, 512], f32, tag="po1")
                    nc.tensor.matmul(
                        out=po1[:],
                        lhsT=mmcast(w3m_sb[:, hs]),
                        rhs=mmcast(mean_n[:, cols]),
                        start=True,
                        stop=False,
                    )
                    nc.tensor.matmul(
                        out=po1[:],
                        lhsT=mmcast(w3x_sb[:, hs]),
                        rhs=mmcast(xT_sb[:, cols]),
                        start=False,
                        stop=True,
                    )
                    t = o1p.tile([128, 512], DT_N, tag=f"o1_{h}")
                    nc.vector.tensor_scalar(
                        out=t[:],
                        in0=po1[:],
                        scalar1=b3_sb[:, h : h + 1],
                        scalar2=0.0,
                        op0=mybir.AluOpType.add,
                        op1=mybir.AluOpType.max,
                    )
                    o1h.append(t)
                for h in range(2):
                    hs = slice(h * 128, (h + 1) * 128)
                    po2 = nps.tile([128, 512], f32, tag="po2")
                    nc.tensor.matmul(
                        out=po2[:],
                        lhsT=mmcast(w4a_sb[:, hs]),
                        rhs=mmcast(o1h[0][:]),
                        start=True,
                        stop=False,
                    )
                    nc.tensor.matmul(
                        out=po2[:],
                        lhsT=mmcast(w4b_sb[:, hs]),
                        rhs=mmcast(o1h[1][:]),
                        start=False,
                        stop=True,
                    )
                    o2t = o2p.tile([128, 512], f32, tag="o2")
                    nc.scalar.activation(
                        out=o2t[:],
                        in_=po2[:],
                        func=mybir.ActivationFunctionType.Relu,
                        bias=b4_sb[:, h : h + 1],
                    )
                    nc.sync.dma_start(out=oT[hs, cols], in_=o2t[:])

    nc.finalize()
    return nc


# ---------------------------------------------------------------------------
# Entry point
# ---------------------------------------------------------------------------

def kernel(x, edge_index, edge_attr, W1, b1, W2, b2, W3, b3, W4, b4,
           edge_prec="bf16", node_prec="f32r"):
    x = np.asarray(x, dtype=np.float32)
    edge_index = np.asarray(edge_index)
    edge_attr = np.asarray(edge_attr, dtype=np.float32)
    W1 = np.asarray(W1, dtype=np.float32)
    b1 = np.asarray(b1, dtype=np.float32)
    W2 = np.asarray(W2, dtype=np.float32)
    b2 = np.asarray(b2, dtype=np.float32)
    W3 = np.asarray(W3, dtype=np.float32)
    b3 = np.asarray(b3, dtype=np.float32)
    W4 = np.asarray(W4, dtype=np.float32)
    b4 = np.asarray(b4, dtype=np.float32)

    row = np.asarray(edge_index[0], dtype=np.int64)
    col = np.asarray(edge_index[1], dtype=np.int64)

    edge_dt = ml_dtypes.bfloat16 if edge_prec == "bf16" else np.float32
    node_dt = ml_dtypes.bfloat16 if node_prec == "bf16" else np.float32
    f32r = node_prec == "f32r" or edge_prec == "f32r"

    in_maps, meta = _preprocess(x, row, col, edge_attr, edge_dt, node_dt)

    # weights: shared across cores
    w1c43 = np.vstack([W1[X_DIM:], W1[:X_DIM]])  # [43, 64]
    w1c = np.zeros((128, H1), dtype=np.float32)
    w1c[:EAX_DIM] = w1c43
    w1c[64 : 64 + EAX_DIM] = w1c43
    w1c = w1c.astype(edge_dt)
    w2d = np.vstack([W2, W2]).astype(edge_dt)  # [128, 128], one copy per pair half
    w3m = W3[X_DIM:].astype(node_dt)
    w3x = W3[:X_DIM].astype(node_dt)
    w4a = W4[:128].astype(node_dt)
    w4b = W4[128:].astype(node_dt)
    b1d = np.concatenate([b1, b1]).reshape(128, 1).astype(np.float32)
    b2d = b2.reshape(H2, 1).astype(np.float32)
    b3d = b3.reshape(2, 128).T.copy().astype(np.float32)
    b4d = b4.reshape(2, 128).T.copy().astype(np.float32)
    for m in in_maps:
        m.update(
            w1c=w1c, w2d=w2d, w3m=w3m, w3x=w3x, w4a=w4a, w4b=w4b,
            b1d=b1d, b2d=b2d, b3d=b3d, b4d=b4d,
        )

    nc = _build_program(
        meta, W1, b1, W2, b2, W3, b3, W4, b4,
        edge_dt, node_dt, matmul_f32r=f32r,
    )

    from concourse.bass_utils import run_bass_kernel_spmd

    res = run_bass_kernel_spmd(nc, in_maps, list(range(N_CORES)), **RUN_KWARGS)
    global LAST_EXEC_NS, LAST_RESULT
    LAST_EXEC_NS = res.exec_time_ns
    LAST_RESULT = res

    out = np.zeros((N_NODES, H3), dtype=np.float32)
    for c in range(N_CORES):
        oT_c = np.asarray(res.results[c]["oT"])  # [256, NS_pad]
        slots = meta["slot_tables"][c]
        valid = slots >= 0
        out[slots[valid]] = oT_c[:, valid].T
    return out


if __name__ == "__main__":
    # tiny self-test with a small synthetic graph via monkeypatched sizes
    pass


# revision 34
# speedup vs baseline: 1.6206x; 1.0209x over previous
"""GNN message-passing kernel for 8 Trainium2 NeuronCores.

Strategy (destination-sharded, degree-sorted):
  - Nodes are assigned to cores round-robin by degree rank, so every core's
    per-degree node counts match (after tiny padding) and one SPMD program
    serves all 8 cores.
  - Each core's edges are laid out grouped by destination node, nodes grouped
    by exact degree d.  The segment-sum over edges then becomes a dense
    [128ch, n_nodes, d] free-axis reduction - no scatter, no one-hot.
  - Edge MLP runs channels-on-partitions: in^T [43, E] tiles, two stationary
    weight matmuls (43->64, 65->128 with bias folded via a ones row).
  - Node MLP runs on the mean slab [128ch, node_slots] directly.
  - Host does index preprocessing, the x[col] gather into the edge-feature
    shard, and the final unpermute.
"""

import math
import os

import numpy as np
import ml_dtypes

N_NODES = 100000
N_EDGES = 1600000
N_CORES = 8
X_DIM, E_DIM = 4, 39
EAX_DIM = X_DIM + E_DIM  # 43
H1, H2 = 64, 128
H3 = 256

ROUND_EDGES = 1024  # compute-round edge budget (two PSUM banks at fp32)
DMA_EDGES = 4096    # edge-feature DMA granularity
RELU2_DVE_FRAC = 0.32  # fraction of second-relu columns evacuated on DVE

RUN_KWARGS: dict = {}
LAST_EXEC_NS = None
LAST_RESULT = None

F32 = "float32"


def _apply_tile_drain_patch():
    """walrus in this env only accepts one sync wait on a TPB_CTRL drain;
    split the Tile tail drain's waits across multiple drain instructions."""
    import bass_rust
    from concourse.tile import TileContext, ScopedClock

    if getattr(TileContext, "_drain_patch_applied", False):
        return

    def _patched(self, tick_clock, wait_clock):
        nc = self.nc
        drain_inst = nc.sync.drain()
        wait_clock.add_sem_waits(
            drain_inst.ins, ScopedClock({None: tick_clock.global_clock})
        )
        si = drain_inst.ins.sync_info
        waits = list(si.on_wait) if si is not None else []
        if len(waits) > 1:
            drain_inst.ins.sync_info = bass_rust.SyncInfo(
                on_wait=[waits[0]], on_update=[]
            )
            for w in waits[1:]:
                d2 = nc.sync.drain()
                d2.ins.sync_info = bass_rust.SyncInfo(on_wait=[w], on_update=[])
        nc.all_engine_barrier()
        assert self.sems is not None
        popped = nc._tile_sem_poison_stack.pop()
        assert popped is self._sem_poison
        nc.clear_and_free_semaphores(list(self.sems.allocated().values()))
        nc.all_engine_barrier()

    TileContext._drain_and_barrier = _patched
    TileContext._drain_patch_applied = True


# ---------------------------------------------------------------------------
# Host-side preprocessing
# ---------------------------------------------------------------------------

def _preprocess(x, row, col, edge_attr, edge_dt, node_dt):
    """Build per-core shards. Returns (in_maps, meta)."""
    deg = np.bincount(row, minlength=N_NODES).astype(np.int64)

    # Degree-ascending node order; node i of the order goes to core i % 8.
    node_order = np.argsort(deg, kind="stable")
    deg_sorted = deg[node_order]

    # Per-degree uniform region sizes M_d = max over cores of per-core count.
    degrees = np.unique(deg_sorted)
    # count of nodes with degree d on core c: split counts of each degree run
    # over cores: run of length L starting at global index s -> core (s+k)%8.
    run_starts = np.searchsorted(deg_sorted, degrees, side="left")
    run_lens = np.searchsorted(deg_sorted, degrees, side="right") - run_starts
    # M_d: ceil division accounting for phase; max over cores is
    # ceil(L/8) when L%8 != 0 aligned anywhere -> just use ceil(L/8) if the
    # run is spread evenly, but phase can make one core get one extra:
    # max count = ceil((L + (s % 8 accounted)) ... simply compute exactly.
    M = np.empty(len(degrees), dtype=np.int64)
    m_dc = np.empty((len(degrees), N_CORES), dtype=np.int64)
    for j, (s, L) in enumerate(zip(run_starts, run_lens)):
        idx = (s + np.arange(L)) % N_CORES
        cnt = np.bincount(idx, minlength=N_CORES)
        m_dc[j] = cnt
        M[j] = cnt.max()

    node_off = np.concatenate([[0], np.cumsum(M)])  # region node-slot offsets
    NS = int(node_off[-1])  # node slots per core (incl. per-degree pads)
    NS_pad = ((NS + 511) // 512) * 512
    edge_off = np.concatenate([[0], np.cumsum(M * degrees)])
    E_TOT = int(edge_off[-1])
    E_TOT_pad = ((E_TOT + DMA_EDGES - 1) // DMA_EDGES) * DMA_EDGES

    # --- compute rounds and DMA chunk packing (shared with the builder) ---
    # round: (edge_start, n_nodes, d, node_slot_start)
    rounds = []
    for j, d in enumerate(degrees):
        if d == 0:
            continue
        m = int(M[j])
        npr = max(1, ROUND_EDGES // int(d))
        s = 0
        while s < m:
            n = min(npr, m - s)
            rounds.append((int(edge_off[j] + s * d), n, int(d), int(node_off[j] + s)))
            s += n
    # chunks of whole rounds, <= DMA_EDGES contiguous edges
    chunks = []  # [c0, clen, [round indices]]
    cur = None
    for ri, (e0, n, d, s0) in enumerate(rounds):
        ln = n * d
        if cur is not None and e0 + ln - cur[0] <= DMA_EDGES:
            cur[1] = e0 + ln - cur[0]
            cur[2].append(ri)
        else:
            if cur is not None:
                chunks.append(cur)
            cur = [e0, ln, [ri]]
    if cur is not None:
        chunks.append(cur)
    # pack chunk pairs into 128-partition column blocks: chunk 2s -> rows
    # [0:43], chunk 2s+1 -> rows [64:107], both at col base scol[s]
    scols = []
    rowbases = []
    base = 0
    for ci, (c0, clen, ris) in enumerate(chunks):
        if ci % 2 == 0:
            scols.append(base)
            rowbases.append(0)
            width = clen
        else:
            scols.append(scols[-1])
            rowbases.append(64)
            width = max(width, clen)
        if ci % 2 == 1 or ci == len(chunks) - 1:
            base += width
    EC = ((base + DMA_EDGES - 1) // DMA_EDGES) * DMA_EDGES  # eax column count

    # --- per-core slot assignment -----------------------------------------
    # nodes of core c in degree order: node_order[c::8] with degree run
    # boundaries; slot of k-th node of degree d on core c = node_off[j] + k.
    in_maps = []
    slot_tables = []  # per core: global node id per slot (-1 pad)
    x = x.astype(np.float32)
    edge_attr = edge_attr.astype(np.float32)
    recip = 1.0 / np.maximum(deg, 1.0)

    # edge -> (core, slot) of its destination
    # global: position of node in sorted order
    pos_of_node = np.empty(N_NODES, dtype=np.int64)
    pos_of_node[node_order] = np.arange(N_NODES)
    core_of_node = pos_of_node % N_CORES

    # per-core, per-degree start index within the core's degree-sorted list
    for c in range(N_CORES):
        nodes_c = node_order[c::N_CORES]  # ascending degree
        deg_c = deg[nodes_c]
        # within-degree rank for this core's nodes
        # nodes_c sorted by degree; run starts:
        starts_c = np.searchsorted(deg_c, degrees, side="left")
        j_of_deg = np.searchsorted(degrees, deg_c)  # region index per node
        rank = np.arange(len(nodes_c)) - starts_c[j_of_deg]
        slot = node_off[j_of_deg] + rank

        slots = np.full(NS_pad, -1, dtype=np.int64)
        slots[slot] = nodes_c
        slot_tables.append(slots)

        # --- edges of this core ------------------------------------------
        emask = core_of_node[row] == c
        er = row[emask]
        ec = col[emask]
        ea = edge_attr[emask]
        # slot of dest node
        slot_of_node = np.full(N_NODES, -1, dtype=np.int64)
        slot_of_node[nodes_c] = slot
        es = slot_of_node[er]
        order = np.argsort(es, kind="stable")
        es_s = es[order]
        # within-node running index
        uniq, first_idx, counts = np.unique(
            es_s, return_index=True, return_counts=True
        )
        within = np.arange(len(es_s)) - np.repeat(first_idx, counts)
        # edge slot base per node slot: edge_off[j] + (slot-node_off[j])*d
        j_of_slot = j_of_deg[np.argsort(slot, kind="stable")]  # slot-> region
        # simpler: recompute region of each sorted edge's dest slot
        j_e = np.searchsorted(node_off[1:], es_s, side="right")
        d_e = degrees[j_e]
        ebase = edge_off[j_e] + (es_s - node_off[j_e]) * d_e
        epos = ebase + within

        chunk_starts = np.array([ch[0] for ch in chunks], dtype=np.int64)
        sc_arr = np.array(scols, dtype=np.int64)
        rb_arr = np.array(rowbases, dtype=np.int64)
        ci_e = np.searchsorted(chunk_starts, epos, side="right") - 1
        cols_e = sc_arr[ci_e] + (epos - chunk_starts[ci_e])
        rb_e = rb_arr[ci_e]
        ea_s = ea[order]
        xg_s = x[ec[order]]
        eax = np.zeros((128, EC), dtype=np.float32)
        for b in (0, 64):
            msk = rb_e == b
            eax[b : b + E_DIM, cols_e[msk]] = ea_s[msk].T
            eax[b + E_DIM : b + EAX_DIM, cols_e[msk]] = xg_s[msk].T

        xT = np.zeros((X_DIM, NS_pad), dtype=np.float32)
        valid = slots >= 0
        xT[:, valid] = x[slots[valid]].T
        rr = np.ones(NS_pad, dtype=np.float32)
        rr[valid] = recip[slots[valid]]
        rrep = np.broadcast_to(rr[None, :], (128, NS_pad)).copy()

        in_maps.append(
            {
                "eax": eax.astype(edge_dt),
                "xT": xT.astype(node_dt),
                "rrep": rrep.astype(edge_dt),
            }
        )

    meta = {
        "NS": NS,
        "NS_pad": NS_pad,
        "E_TOT": E_TOT,
        "EC": EC,
        "rounds": rounds,
        "chunks": chunks,
        "scols": scols,
        "rowbases": rowbases,
        "slot_tables": slot_tables,
    }
    return in_maps, meta


# ---------------------------------------------------------------------------
# Device program
# ---------------------------------------------------------------------------

def _build_program(meta, W1, b1, W2, b2, W3, b3, W4, b4, edge_dt, node_dt,
                   matmul_f32r=True):
    import concourse.bass as bass
    import concourse.mybir as mybir
    from concourse import bacc
    from concourse.tile import TileContext

    _apply_tile_drain_patch()

    DT_E = mybir.dt.bfloat16 if edge_dt == ml_dtypes.bfloat16 else mybir.dt.float32
    DT_N = mybir.dt.bfloat16 if node_dt == ml_dtypes.bfloat16 else mybir.dt.float32
    f32 = mybir.dt.float32

    NS_pad = meta["NS_pad"]
    EC = meta["EC"]
    rounds = meta["rounds"]
    chunks = meta["chunks"]
    scols = meta["scols"]
    rowbases = meta["rowbases"]

    nc = bacc.Bacc("TRN2", target_bir_lowering=False, debug=False)
    eax = nc.declare_dram_parameter("eax", [128, EC], DT_E, isOutput=False)
    xT = nc.declare_dram_parameter("xT", [X_DIM, NS_pad], DT_N, isOutput=False)
    rrep = nc.declare_dram_parameter("rrep", [128, NS_pad], DT_E, isOutput=False)
    w1c = nc.declare_dram_parameter("w1c", [128, H1], DT_E, isOutput=False)
    b1d = nc.declare_dram_parameter("b1d", [128, 1], f32, isOutput=False)
    w2d = nc.declare_dram_parameter("w2d", [2 * H1, H2], DT_E, isOutput=False)
    b2d = nc.declare_dram_parameter("b2d", [H2, 1], f32, isOutput=False)
    w3m = nc.declare_dram_parameter("w3m", [H2, H3], DT_N, isOutput=False)
    w3x = nc.declare_dram_parameter("w3x", [X_DIM, H3], DT_N, isOutput=False)
    w4a = nc.declare_dram_parameter("w4a", [128, H3], DT_N, isOutput=False)
    w4b = nc.declare_dram_parameter("w4b", [128, H3], DT_N, isOutput=False)
    b3d = nc.declare_dram_parameter("b3d", [128, 2], f32, isOutput=False)
    b4d = nc.declare_dram_parameter("b4d", [128, 2], f32, isOutput=False)
    oT = nc.declare_dram_parameter("oT", [H3, NS_pad], f32, isOutput=True)

    def mmcast(ap):
        if matmul_f32r and ap.dtype == mybir.dt.float32:
            return ap.bitcast(mybir.dt.float32r)
        return ap

    with TileContext(nc) as tc:
      with tc.tile_pool(name="slab", bufs=1) as slabp:
        mean_slab = slabp.tile([128, NS_pad], DT_E)
        with (
            tc.tile_pool(name="const", bufs=1) as constp,
            tc.tile_pool(name="eaxp", bufs=3) as eaxp,
            tc.tile_pool(name="h1p", bufs=3) as h1p,
            tc.tile_pool(name="h2p", bufs=3) as h2p,
            tc.tile_pool(name="ph1p", bufs=2, space="PSUM") as ph1p,
            tc.tile_pool(name="ph2p", bufs=2, space="PSUM") as ph2p,
        ):
            w1c_sb = constp.tile([128, H1], DT_E)
            nc.gpsimd.dma_start(out=w1c_sb[:], in_=w1c[:])
            w2_sb = constp.tile([2 * H1, H2], DT_E)
            nc.gpsimd.dma_start(out=w2_sb[:], in_=w2d[:])
            b1_sb = constp.tile([128, 1], f32)  # b1 stacked twice (pairing)
            nc.gpsimd.dma_start(out=b1_sb[:], in_=b1d[:])
            b2_sb = constp.tile([H2, 1], f32)
            nc.gpsimd.dma_start(out=b2_sb[:], in_=b2d[:])

            nc.gpsimd.memset(mean_slab[:], 0.0)

            def do_relu2_and_reduce(ph2t, re, n, d, s0):
                h2t = h2p.tile([H2, ROUND_EDGES], DT_E, tag="h2")
                ca = re - int(re * RELU2_DVE_FRAC)
                if ca > 0:
                    nc.scalar.activation(
                        out=h2t[:, :ca],
                        in_=ph2t[:, :ca],
                        func=mybir.ActivationFunctionType.Relu,
                        bias=b2_sb[:],
                    )
                if re - ca > 0:
                    nc.vector.tensor_scalar(
                        out=h2t[:, ca:re],
                        in0=ph2t[:, ca:re],
                        scalar1=b2_sb[:],
                        scalar2=0.0,
                        op0=mybir.AluOpType.add,
                        op1=mybir.AluOpType.max,
                    )
                src = h2t[:, :re].rearrange("p (n d) -> p n d", d=d)
                with nc.allow_low_precision(reason="bf16 mean slab"):
                    nc.vector.tensor_reduce(
                        out=mean_slab[:, s0 : s0 + n],
                        in_=src,
                        op=mybir.AluOpType.add,
                        axis=mybir.AxisListType.X,
                    )

            for ci, (c0, clen, ris) in enumerate(chunks):
                if rowbases[ci] == 0:
                    w = clen
                    if ci + 1 < len(chunks) and rowbases[ci + 1] == 64:
                        w = max(w, chunks[ci + 1][1])
                    eax_t = eaxp.tile([128, DMA_EDGES], DT_E, tag="eax")
                    nc.sync.dma_start(
                        out=eax_t[:, :w], in_=eax[:, scols[ci] : scols[ci] + w]
                    )
                rb = rowbases[ci]
                for k in range(0, len(ris), 2):
                    pair = ris[k : k + 2]
                    rs = [rounds[ri] for ri in pair]
                    res = [n * d for (_, n, d, _) in rs]
                    remax = max(res)
                    ph1 = ph1p.tile([128, ROUND_EDGES], f32, tag="ph1")
                    for half, ((e0, n, d, s0), re) in enumerate(zip(rs, res)):
                        for j in range(0, re, 512):
                            wj = min(512, re - j)
                            ro = e0 - c0
                            nc.tensor.matmul(
                                out=ph1[half * H1 : half * H1 + H1, j : j + wj],
                                lhsT=mmcast(w1c_sb[rb : rb + EAX_DIM, :]),
                                rhs=mmcast(eax_t[rb : rb + EAX_DIM, ro + j : ro + j + wj]),
                                start=True,
                                stop=True,
                            )
                    h1t = h1p.tile([128, ROUND_EDGES], DT_E, tag="h1")
                    nparts = 128 if len(rs) == 2 else H1
                    nc.scalar.activation(
                        out=h1t[:nparts, :remax],
                        in_=ph1[:nparts, :remax],
                        func=mybir.ActivationFunctionType.Relu,
                        bias=b1_sb[:nparts],
                    )
                    for half, ((e0, n, d, s0), re) in enumerate(zip(rs, res)):
                        ph2 = ph2p.tile([H2, ROUND_EDGES], f32, tag="ph2")
                        for j in range(0, re, 512):
                            w = min(512, re - j)
                            nc.tensor.matmul(
                                out=ph2[:, j : j + w],
                                lhsT=mmcast(w2_sb[half * H1 : (half + 1) * H1, :]),
                                rhs=mmcast(
                                    h1t[half * H1 : half * H1 + H1, j : j + w]
                                ),
                                start=True,
                                stop=True,
                            )
                        if d > ROUND_EDGES:
                            raise NotImplementedError("degree > ROUND_EDGES")
                        do_relu2_and_reduce(ph2[:], re, n, d, s0)

            # mean scaling by 1/deg
            rrep_sb = constp.tile([128, NS_pad], DT_E)
            nc.gpsimd.dma_start(out=rrep_sb[:], in_=rrep[:])
            with nc.allow_low_precision(reason="bf16 mean slab"):
                for s in range(0, NS_pad, 2048):
                    e = min(s + 2048, NS_pad)
                    nc.vector.tensor_tensor(
                        out=mean_slab[:, s:e],
                        in0=mean_slab[:, s:e],
                        in1=rrep_sb[:, s:e],
                        op=mybir.AluOpType.mult,
                    )

        # ---- node MLP phase ----
        with (
            tc.tile_pool(name="nconst", bufs=1) as nconstp,
            tc.tile_pool(name="o1p", bufs=4) as o1p,
            tc.tile_pool(name="o2p", bufs=3) as o2p,
            tc.tile_pool(name="nps", bufs=4, space="PSUM") as nps,
        ):
            w3m_sb = nconstp.tile([H2, H3], DT_N)
            nc.gpsimd.dma_start(out=w3m_sb[:], in_=w3m[:])
            w3x_sb = nconstp.tile([X_DIM, H3], DT_N)
            nc.gpsimd.dma_start(out=w3x_sb[:], in_=w3x[:])
            w4a_sb = nconstp.tile([128, H3], DT_N)
            nc.gpsimd.dma_start(out=w4a_sb[:], in_=w4a[:])
            w4b_sb = nconstp.tile([128, H3], DT_N)
            nc.gpsimd.dma_start(out=w4b_sb[:], in_=w4b[:])
            b3_sb = nconstp.tile([128, 2], f32)
            nc.gpsimd.dma_start(out=b3_sb[:], in_=b3d[:])
            b4_sb = nconstp.tile([128, 2], f32)
            nc.gpsimd.dma_start(out=b4_sb[:], in_=b4d[:])
            xT_sb = nconstp.tile([X_DIM, NS_pad], DT_N)
            nc.gpsimd.dma_start(out=xT_sb[:], in_=xT[:])

            # mean slab may need dtype cast for node matmuls
            if DT_N != DT_E:
                mean_n = nconstp.tile([128, NS_pad], DT_N)
                for s in range(0, NS_pad, 2048):
                    e = min(s + 2048, NS_pad)
                    nc.vector.tensor_copy(
                        out=mean_n[:, s:e], in_=mean_slab[:, s:e]
                    )
            else:
                mean_n = mean_slab

            for blk in range(NS_pad // 512):
                cols = slice(blk * 512, (blk + 1) * 512)
                o1h = []
                for h in range(2):
                    hs = slice(h * 128, (h + 1) * 128)
                    po1 = nps.tile([128, 512], f32, tag="po1")
                    nc.tensor.matmul(
                        out=po1[:],
                        lhsT=mmcast(w3m_sb[:, hs]),
                        rhs=mmcast(mean_n[:, cols]),
                        start=True,
                        stop=False,
                    )
                    nc.tensor.matmul(
                        out=po1[:],
                        lhsT=mmcast(w3x_sb[:, hs]),
                        rhs=mmcast(xT_sb[:, cols]),
                        start=False,
                        stop=True,
                    )
                    t = o1p.tile([128, 512], DT_N, tag=f"o1_{h}")
                    nc.vector.tensor_scalar(
                        out=t[:],
                        in0=po1[:],
                        scalar1=b3_sb[:, h : h + 1],
                        scalar2=0.0,
                        op0=mybir.AluOpType.add,
                        op1=mybir.AluOpType.max,
                    )
                    o1h.append(t)
                for h in range(2):
                    hs = slice(h * 128, (h + 1) * 128)
                    po2 = nps.tile([128, 512], f32, tag="po2")
                    nc.tensor.matmul(
                        out=po2[:],
                        lhsT=mmcast(w4a_sb[:, hs]),
                        rhs=mmcast(o1h[0][:]),
                        start=True,
                        stop=False,
                    )
                    nc.tensor.matmul(
                        out=po2[:],
                        lhsT=mmcast(w4b_sb[:, hs]),
                        rhs=mmcast(o1h[1][:]),
                        start=False,
                        stop=True,
                    )
                    o2t = o2p.tile([128, 512], f32, tag="o2")
                    nc.scalar.activation(
                        out=o2t[:],
                        in_=po2[:],
                        func=mybir.ActivationFunctionType.Relu,
                        bias=b4_sb[:, h : h + 1],
                    )
                    nc.sync.dma_start(out=oT[hs, cols], in_=o2t[:])

    nc.finalize()
    return nc


# ---------------------------------------------------------------------------
# Entry point
# ---------------------------------------------------------------------------

def kernel(x, edge_index, edge_attr, W1, b1, W2, b2, W3, b3, W4, b4,
           edge_prec="bf16", node_prec="f32r"):
    x = np.asarray(x, dtype=np.float32)
    edge_index = np.asarray(edge_index)
    edge_attr = np.asarray(edge_attr, dtype=np.float32)
    W1 = np.asarray(W1, dtype=np.float32)
    b1 = np.asarray(b1, dtype=np.float32)
    W2 = np.asarray(W2, dtype=np.float32)
    b2 = np.asarray(b2, dtype=np.float32)
    W3 = np.asarray(W3, dtype=np.float32)
    b3 = np.asarray(b3, dtype=np.float32)
    W4 = np.asarray(W4, dtype=np.float32)
    b4 = np.asarray(b4, dtype=np.float32)

    row = np.asarray(edge_index[0], dtype=np.int64)
    col = np.asarray(edge_index[1], dtype=np.int64)

    edge_dt = ml_dtypes.bfloat16 if edge_prec == "bf16" else np.float32
    node_dt = ml_dtypes.bfloat16 if node_prec == "bf16" else np.float32
    f32r = node_prec == "f32r" or edge_prec == "f32r"

    in_maps, meta = _preprocess(x, row, col, edge_attr, edge_dt, node_dt)

    # weights: shared across cores
    w1c43 = np.vstack([W1[X_DIM:], W1[:X_DIM]])  # [43, 64]
    w1c = np.zeros((128, H1), dtype=np.float32)
    w1c[:EAX_DIM] = w1c43
    w1c[64 : 64 + EAX_DIM] = w1c43
    w1c = w1c.astype(edge_dt)
    w2d = np.vstack([W2, W2]).astype(edge_dt)  # [128, 128], one copy per pair half
    w3m = W3[X_DIM:].astype(node_dt)
    w3x = W3[:X_DIM].astype(node_dt)
    w4a = W4[:128].astype(node_dt)
    w4b = W4[128:].astype(node_dt)
    b1d = np.concatenate([b1, b1]).reshape(128, 1).astype(np.float32)
    b2d = b2.reshape(H2, 1).astype(np.float32)
    b3d = b3.reshape(2, 128).T.copy().astype(np.float32)
    b4d = b4.reshape(2, 128).T.copy().astype(np.float32)
    for m in in_maps:
        m.update(
            w1c=w1c, w2d=w2d, w3m=w3m, w3x=w3x, w4a=w4a, w4b=w4b,
            b1d=b1d, b2d=b2d, b3d=b3d, b4d=b4d,
        )

    nc = _build_program(
        meta, W1, b1, W2, b2, W3, b3, W4, b4,
        edge_dt, node_dt, matmul_f32r=f32r,
    )

    from concourse.bass_utils import run_bass_kernel_spmd

    res = run_bass_kernel_spmd(nc, in_maps, list(range(N_CORES)), **RUN_KWARGS)
    global LAST_EXEC_NS, LAST_RESULT
    LAST_EXEC_NS = res.exec_time_ns
    LAST_RESULT = res

    out = np.zeros((N_NODES, H3), dtype=np.float32)
    for c in range(N_CORES):
        oT_c = np.asarray(res.results[c]["oT"])  # [256, NS_pad]
        slots = meta["slot_tables"][c]
        valid = slots >= 0
        out[slots[valid]] = oT_c[:, valid].T
    return out


if __name__ == "__main__":
    # tiny self-test with a small synthetic graph via monkeypatched sizes
    pass


# revision 36
# speedup vs baseline: 1.6358x; 1.0094x over previous
"""GNN message-passing kernel for 8 Trainium2 NeuronCores.

Strategy (destination-sharded, degree-sorted):
  - Nodes are assigned to cores round-robin by degree rank, so every core's
    per-degree node counts match (after tiny padding) and one SPMD program
    serves all 8 cores.
  - Each core's edges are laid out grouped by destination node, nodes grouped
    by exact degree d.  The segment-sum over edges then becomes a dense
    [128ch, n_nodes, d] free-axis reduction - no scatter, no one-hot.
  - Edge MLP runs channels-on-partitions: in^T [43, E] tiles, two stationary
    weight matmuls (43->64, 65->128 with bias folded via a ones row).
  - Node MLP runs on the mean slab [128ch, node_slots] directly.
  - Host does index preprocessing, the x[col] gather into the edge-feature
    shard, and the final unpermute.
"""

import math
import os

import numpy as np
import ml_dtypes

N_NODES = 100000
N_EDGES = 1600000
N_CORES = 8
X_DIM, E_DIM = 4, 39
EAX_DIM = X_DIM + E_DIM  # 43
H1, H2 = 64, 128
H3 = 256

ROUND_EDGES = 1024  # compute-round edge budget (two PSUM banks at fp32)
DMA_EDGES = 4096    # edge-feature DMA granularity
RELU2_DVE_FRAC = 0.30  # fraction of second-relu columns evacuated on DVE

RUN_KWARGS: dict = {}
LAST_EXEC_NS = None
LAST_RESULT = None

F32 = "float32"


def _apply_tile_drain_patch():
    """walrus in this env only accepts one sync wait on a TPB_CTRL drain;
    split the Tile tail drain's waits across multiple drain instructions."""
    import bass_rust
    from concourse.tile import TileContext, ScopedClock

    if getattr(TileContext, "_drain_patch_applied", False):
        return

    def _patched(self, tick_clock, wait_clock):
        nc = self.nc
        drain_inst = nc.sync.drain()
        wait_clock.add_sem_waits(
            drain_inst.ins, ScopedClock({None: tick_clock.global_clock})
        )
        si = drain_inst.ins.sync_info
        waits = list(si.on_wait) if si is not None else []
        if len(waits) > 1:
            drain_inst.ins.sync_info = bass_rust.SyncInfo(
                on_wait=[waits[0]], on_update=[]
            )
            for w in waits[1:]:
                d2 = nc.sync.drain()
                d2.ins.sync_info = bass_rust.SyncInfo(on_wait=[w], on_update=[])
        nc.all_engine_barrier()
        assert self.sems is not None
        popped = nc._tile_sem_poison_stack.pop()
        assert popped is self._sem_poison
        nc.clear_and_free_semaphores(list(self.sems.allocated().values()))
        nc.all_engine_barrier()

    TileContext._drain_and_barrier = _patched
    TileContext._drain_patch_applied = True


# ---------------------------------------------------------------------------
# Host-side preprocessing
# ---------------------------------------------------------------------------

def _preprocess(x, row, col, edge_attr, edge_dt, node_dt):
    """Build per-core shards. Returns (in_maps, meta)."""
    deg = np.bincount(row, minlength=N_NODES).astype(np.int64)

    # Degree-ascending node order; node i of the order goes to core i % 8.
    node_order = np.argsort(deg, kind="stable")
    deg_sorted = deg[node_order]

    # Per-degree uniform region sizes M_d = max over cores of per-core count.
    degrees = np.unique(deg_sorted)
    # count of nodes with degree d on core c: split counts of each degree run
    # over cores: run of length L starting at global index s -> core (s+k)%8.
    run_starts = np.searchsorted(deg_sorted, degrees, side="left")
    run_lens = np.searchsorted(deg_sorted, degrees, side="right") - run_starts
    # M_d: ceil division accounting for phase; max over cores is
    # ceil(L/8) when L%8 != 0 aligned anywhere -> just use ceil(L/8) if the
    # run is spread evenly, but phase can make one core get one extra:
    # max count = ceil((L + (s % 8 accounted)) ... simply compute exactly.
    M = np.empty(len(degrees), dtype=np.int64)
    m_dc = np.empty((len(degrees), N_CORES), dtype=np.int64)
    for j, (s, L) in enumerate(zip(run_starts, run_lens)):
        idx = (s + np.arange(L)) % N_CORES
        cnt = np.bincount(idx, minlength=N_CORES)
        m_dc[j] = cnt
        M[j] = cnt.max()

    node_off = np.concatenate([[0], np.cumsum(M)])  # region node-slot offsets
    NS = int(node_off[-1])  # node slots per core (incl. per-degree pads)
    NS_pad = ((NS + 511) // 512) * 512
    edge_off = np.concatenate([[0], np.cumsum(M * degrees)])
    E_TOT = int(edge_off[-1])
    E_TOT_pad = ((E_TOT + DMA_EDGES - 1) // DMA_EDGES) * DMA_EDGES

    # --- compute rounds and DMA chunk packing (shared with the builder) ---
    # round: (edge_start, n_nodes, d, node_slot_start)
    rounds = []
    for j, d in enumerate(degrees):
        if d == 0:
            continue
        m = int(M[j])
        npr = max(1, ROUND_EDGES // int(d))
        s = 0
        while s < m:
            n = min(npr, m - s)
            rounds.append((int(edge_off[j] + s * d), n, int(d), int(node_off[j] + s)))
            s += n
    # chunks of whole rounds, <= DMA_EDGES contiguous edges
    chunks = []  # [c0, clen, [round indices]]
    cur = None
    for ri, (e0, n, d, s0) in enumerate(rounds):
        ln = n * d
        if cur is not None and e0 + ln - cur[0] <= DMA_EDGES:
            cur[1] = e0 + ln - cur[0]
            cur[2].append(ri)
        else:
            if cur is not None:
                chunks.append(cur)
            cur = [e0, ln, [ri]]
    if cur is not None:
        chunks.append(cur)
    # pack chunk pairs into 128-partition column blocks: chunk 2s -> rows
    # [0:43], chunk 2s+1 -> rows [64:107], both at col base scol[s]
    scols = []
    rowbases = []
    base = 0
    for ci, (c0, clen, ris) in enumerate(chunks):
        if ci % 2 == 0:
            scols.append(base)
            rowbases.append(0)
            width = clen
        else:
            scols.append(scols[-1])
            rowbases.append(64)
            width = max(width, clen)
        if ci % 2 == 1 or ci == len(chunks) - 1:
            base += width
    EC = ((base + DMA_EDGES - 1) // DMA_EDGES) * DMA_EDGES  # eax column count

    # --- per-core slot assignment -----------------------------------------
    # nodes of core c in degree order: node_order[c::8] with degree run
    # boundaries; slot of k-th node of degree d on core c = node_off[j] + k.
    in_maps = []
    slot_tables = []  # per core: global node id per slot (-1 pad)
    x = x.astype(np.float32)
    edge_attr = edge_attr.astype(np.float32)
    recip = 1.0 / np.maximum(deg, 1.0)

    # edge -> (core, slot) of its destination
    # global: position of node in sorted order
    pos_of_node = np.empty(N_NODES, dtype=np.int64)
    pos_of_node[node_order] = np.arange(N_NODES)
    core_of_node = pos_of_node % N_CORES

    # per-core, per-degree start index within the core's degree-sorted list
    for c in range(N_CORES):
        nodes_c = node_order[c::N_CORES]  # ascending degree
        deg_c = deg[nodes_c]
        # within-degree rank for this core's nodes
        # nodes_c sorted by degree; run starts:
        starts_c = np.searchsorted(deg_c, degrees, side="left")
        j_of_deg = np.searchsorted(degrees, deg_c)  # region index per node
        rank = np.arange(len(nodes_c)) - starts_c[j_of_deg]
        slot = node_off[j_of_deg] + rank

        slots = np.full(NS_pad, -1, dtype=np.int64)
        slots[slot] = nodes_c
        slot_tables.append(slots)

        # --- edges of this core ------------------------------------------
        emask = core_of_node[row] == c
        er = row[emask]
        ec = col[emask]
        ea = edge_attr[emask]
        # slot of dest node
        slot_of_node = np.full(N_NODES, -1, dtype=np.int64)
        slot_of_node[nodes_c] = slot
        es = slot_of_node[er]
        order = np.argsort(es, kind="stable")
        es_s = es[order]
        # within-node running index
        uniq, first_idx, counts = np.unique(
            es_s, return_index=True, return_counts=True
        )
        within = np.arange(len(es_s)) - np.repeat(first_idx, counts)
        # edge slot base per node slot: edge_off[j] + (slot-node_off[j])*d
        j_of_slot = j_of_deg[np.argsort(slot, kind="stable")]  # slot-> region
        # simpler: recompute region of each sorted edge's dest slot
        j_e = np.searchsorted(node_off[1:], es_s, side="right")
        d_e = degrees[j_e]
        ebase = edge_off[j_e] + (es_s - node_off[j_e]) * d_e
        epos = ebase + within

        chunk_starts = np.array([ch[0] for ch in chunks], dtype=np.int64)
        sc_arr = np.array(scols, dtype=np.int64)
        rb_arr = np.array(rowbases, dtype=np.int64)
        ci_e = np.searchsorted(chunk_starts, epos, side="right") - 1
        cols_e = sc_arr[ci_e] + (epos - chunk_starts[ci_e])
        rb_e = rb_arr[ci_e]
        ea_s = ea[order]
        xg_s = x[ec[order]]
        eax = np.zeros((128, EC), dtype=np.float32)
        for b in (0, 64):
            msk = rb_e == b
            eax[b : b + E_DIM, cols_e[msk]] = ea_s[msk].T
            eax[b + E_DIM : b + EAX_DIM, cols_e[msk]] = xg_s[msk].T

        xT = np.zeros((X_DIM, NS_pad), dtype=np.float32)
        valid = slots >= 0
        xT[:, valid] = x[slots[valid]].T
        rr = np.ones(NS_pad, dtype=np.float32)
        rr[valid] = recip[slots[valid]]
        rrep = np.broadcast_to(rr[None, :], (128, NS_pad)).copy()

        in_maps.append(
            {
                "eax": eax.astype(edge_dt),
                "xT": xT.astype(node_dt),
                "rrep": rrep.astype(edge_dt),
            }
        )

    meta = {
        "NS": NS,
        "NS_pad": NS_pad,
        "E_TOT": E_TOT,
        "EC": EC,
        "rounds": rounds,
        "chunks": chunks,
        "scols": scols,
        "rowbases": rowbases,
        "slot_tables": slot_tables,
    }
    return in_maps, meta


# ---------------------------------------------------------------------------
# Device program
# ---------------------------------------------------------------------------

def _build_program(meta, W1, b1, W2, b2, W3, b3, W4, b4, edge_dt, node_dt,
                   matmul_f32r=True):
    import concourse.bass as bass
    import concourse.mybir as mybir
    from concourse import bacc
    from concourse.tile import TileContext

    _apply_tile_drain_patch()

    DT_E = mybir.dt.bfloat16 if edge_dt == ml_dtypes.bfloat16 else mybir.dt.float32
    DT_N = mybir.dt.bfloat16 if node_dt == ml_dtypes.bfloat16 else mybir.dt.float32
    f32 = mybir.dt.float32

    NS_pad = meta["NS_pad"]
    EC = meta["EC"]
    rounds = meta["rounds"]
    chunks = meta["chunks"]
    scols = meta["scols"]
    rowbases = meta["rowbases"]

    nc = bacc.Bacc("TRN2", target_bir_lowering=False, debug=False)
    eax = nc.declare_dram_parameter("eax", [128, EC], DT_E, isOutput=False)
    xT = nc.declare_dram_parameter("xT", [X_DIM, NS_pad], DT_N, isOutput=False)
    rrep = nc.declare_dram_parameter("rrep", [128, NS_pad], DT_E, isOutput=False)
    w1c = nc.declare_dram_parameter("w1c", [128, H1], DT_E, isOutput=False)
    b1d = nc.declare_dram_parameter("b1d", [128, 1], f32, isOutput=False)
    w2d = nc.declare_dram_parameter("w2d", [2 * H1, H2], DT_E, isOutput=False)
    b2d = nc.declare_dram_parameter("b2d", [H2, 1], f32, isOutput=False)
    w3m = nc.declare_dram_parameter("w3m", [H2, H3], DT_N, isOutput=False)
    w3x = nc.declare_dram_parameter("w3x", [X_DIM, H3], DT_N, isOutput=False)
    w4a = nc.declare_dram_parameter("w4a", [128, H3], DT_N, isOutput=False)
    w4b = nc.declare_dram_parameter("w4b", [128, H3], DT_N, isOutput=False)
    b3d = nc.declare_dram_parameter("b3d", [128, 2], f32, isOutput=False)
    b4d = nc.declare_dram_parameter("b4d", [128, 2], f32, isOutput=False)
    oT = nc.declare_dram_parameter("oT", [H3, NS_pad], f32, isOutput=True)

    def mmcast(ap):
        if matmul_f32r and ap.dtype == mybir.dt.float32:
            return ap.bitcast(mybir.dt.float32r)
        return ap

    with TileContext(nc) as tc:
      with tc.tile_pool(name="slab", bufs=1) as slabp:
        mean_slab = slabp.tile([128, NS_pad], DT_E)
        with (
            tc.tile_pool(name="const", bufs=1) as constp,
            tc.tile_pool(name="eaxp", bufs=4) as eaxp,
            tc.tile_pool(name="h1p", bufs=4) as h1p,
            tc.tile_pool(name="h2p", bufs=4) as h2p,
            tc.tile_pool(name="ph1p", bufs=1, space="PSUM") as ph1p,
            tc.tile_pool(name="ph2p", bufs=3, space="PSUM") as ph2p,
        ):
            w1c_sb = constp.tile([128, H1], DT_E)
            nc.gpsimd.dma_start(out=w1c_sb[:], in_=w1c[:])
            w2_sb = constp.tile([2 * H1, H2], DT_E)
            nc.gpsimd.dma_start(out=w2_sb[:], in_=w2d[:])
            b1_sb = constp.tile([128, 1], f32)  # b1 stacked twice (pairing)
            nc.gpsimd.dma_start(out=b1_sb[:], in_=b1d[:])
            b2_sb = constp.tile([H2, 1], f32)
            nc.gpsimd.dma_start(out=b2_sb[:], in_=b2d[:])

            nc.gpsimd.memset(mean_slab[:], 0.0)

            def do_relu2_and_reduce(ph2t, re, n, d, s0):
                h2t = h2p.tile([H2, ROUND_EDGES], DT_E, tag="h2")
                ca = re - int(re * RELU2_DVE_FRAC)
                if ca > 0:
                    nc.scalar.activation(
                        out=h2t[:, :ca],
                        in_=ph2t[:, :ca],
                        func=mybir.ActivationFunctionType.Relu,
                        bias=b2_sb[:],
                    )
                if re - ca > 0:
                    nc.vector.tensor_scalar(
                        out=h2t[:, ca:re],
                        in0=ph2t[:, ca:re],
                        scalar1=b2_sb[:],
                        scalar2=0.0,
                        op0=mybir.AluOpType.add,
                        op1=mybir.AluOpType.max,
                    )
                src = h2t[:, :re].rearrange("p (n d) -> p n d", d=d)
                with nc.allow_low_precision(reason="bf16 mean slab"):
                    nc.vector.tensor_reduce(
                        out=mean_slab[:, s0 : s0 + n],
                        in_=src,
                        op=mybir.AluOpType.add,
                        axis=mybir.AxisListType.X,
                    )

            for ci, (c0, clen, ris) in enumerate(chunks):
                if rowbases[ci] == 0:
                    w = clen
                    if ci + 1 < len(chunks) and rowbases[ci + 1] == 64:
                        w = max(w, chunks[ci + 1][1])
                    eax_t = eaxp.tile([128, DMA_EDGES], DT_E, tag="eax")
                    nc.sync.dma_start(
                        out=eax_t[:, :w], in_=eax[:, scols[ci] : scols[ci] + w]
                    )
                rb = rowbases[ci]
                for k in range(0, len(ris), 2):
                    pair = ris[k : k + 2]
                    rs = [rounds[ri] for ri in pair]
                    res = [n * d for (_, n, d, _) in rs]
                    remax = max(res)
                    ph1 = ph1p.tile([128, ROUND_EDGES], f32, tag="ph1")
                    for half, ((e0, n, d, s0), re) in enumerate(zip(rs, res)):
                        for j in range(0, re, 512):
                            wj = min(512, re - j)
                            ro = e0 - c0
                            nc.tensor.matmul(
                                out=ph1[half * H1 : half * H1 + H1, j : j + wj],
                                lhsT=mmcast(w1c_sb[rb : rb + EAX_DIM, :]),
                                rhs=mmcast(eax_t[rb : rb + EAX_DIM, ro + j : ro + j + wj]),
                                start=True,
                                stop=True,
                            )
                    h1t = h1p.tile([128, ROUND_EDGES], DT_E, tag="h1")
                    nparts = 128 if len(rs) == 2 else H1
                    nc.scalar.activation(
                        out=h1t[:nparts, :remax],
                        in_=ph1[:nparts, :remax],
                        func=mybir.ActivationFunctionType.Relu,
                        bias=b1_sb[:nparts],
                    )
                    for half, ((e0, n, d, s0), re) in enumerate(zip(rs, res)):
                        ph2 = ph2p.tile([H2, ROUND_EDGES], f32, tag="ph2")
                        for j in range(0, re, 512):
                            w = min(512, re - j)
                            nc.tensor.matmul(
                                out=ph2[:, j : j + w],
                                lhsT=mmcast(w2_sb[half * H1 : (half + 1) * H1, :]),
                                rhs=mmcast(
                                    h1t[half * H1 : half * H1 + H1, j : j + w]
                                ),
                                start=True,
                                stop=True,
                            )
                        if d > ROUND_EDGES:
                            raise NotImplementedError("degree > ROUND_EDGES")
                        do_relu2_and_reduce(ph2[:], re, n, d, s0)

            # mean scaling by 1/deg
            rrep_sb = constp.tile([128, NS_pad], DT_E)
            nc.gpsimd.dma_start(out=rrep_sb[:], in_=rrep[:])
            with nc.allow_low_precision(reason="bf16 mean slab"):
                for s in range(0, NS_pad, 2048):
                    e = min(s + 2048, NS_pad)
                    nc.vector.tensor_tensor(
                        out=mean_slab[:, s:e],
                        in0=mean_slab[:, s:e],
                        in1=rrep_sb[:, s:e],
                        op=mybir.AluOpType.mult,
                    )

        # ---- node MLP phase ----
        with (
            tc.tile_pool(name="nconst", bufs=1) as nconstp,
            tc.tile_pool(name="o1p", bufs=4) as o1p,
            tc.tile_pool(name="o2p", bufs=3) as o2p,
            tc.tile_pool(name="nps", bufs=4, space="PSUM") as nps,
        ):
            w3m_sb = nconstp.tile([H2, H3], DT_N)
            nc.gpsimd.dma_start(out=w3m_sb[:], in_=w3m[:])
            w3x_sb = nconstp.tile([X_DIM, H3], DT_N)
            nc.gpsimd.dma_start(out=w3x_sb[:], in_=w3x[:])
            w4a_sb = nconstp.tile([128, H3], DT_N)
            nc.gpsimd.dma_start(out=w4a_sb[:], in_=w4a[:])
            w4b_sb = nconstp.tile([128, H3], DT_N)
            nc.gpsimd.dma_start(out=w4b_sb[:], in_=w4b[:])
            b3_sb = nconstp.tile([128, 2], f32)
            nc.gpsimd.dma_start(out=b3_sb[:], in_=b3d[:])
            b4_sb = nconstp.tile([128, 2], f32)
            nc.gpsimd.dma_start(out=b4_sb[:], in_=b4d[:])
            xT_sb = nconstp.tile([X_DIM, NS_pad], DT_N)
            nc.gpsimd.dma_start(out=xT_sb[:], in_=xT[:])

            # mean slab may need dtype cast for node matmuls
            if DT_N != DT_E:
                mean_n = nconstp.tile([128, NS_pad], DT_N)
                for s in range(0, NS_pad, 2048):
                    e = min(s + 2048, NS_pad)
                    nc.vector.tensor_copy(
                        out=mean_n[:, s:e], in_=mean_slab[:, s:e]
                    )
            else:
                mean_n = mean_slab

            for blk in range(NS_pad // 512):
                cols = slice(blk * 512, (blk + 1) * 512)
                o1h = []
                for h in range(2):
                    hs = slice(h * 128, (h + 1) * 128)
                    po1 = nps.tile([128, 512], f32, tag="po1")
                    nc.tensor.matmul(
                        out=po1[:],
                        lhsT=mmcast(w3m_sb[:, hs]),
                        rhs=mmcast(mean_n[:, cols]),
                        start=True,
                        stop=False,
                    )
                    nc.tensor.matmul(
                        out=po1[:],
                        lhsT=mmcast(w3x_sb[:, hs]),
                        rhs=mmcast(xT_sb[:, cols]),
                        start=False,
                        stop=True,
                    )
                    t = o1p.tile([128, 512], DT_N, tag=f"o1_{h}")
                    nc.vector.tensor_scalar(
                        out=t[:],
                        in0=po1[:],
                        scalar1=b3_sb[:, h : h + 1],
                        scalar2=0.0,
                        op0=mybir.AluOpType.add,
                        op1=mybir.AluOpType.max,
                    )
                    o1h.append(t)
                for h in range(2):
                    hs = slice(h * 128, (h + 1) * 128)
                    po2 = nps.tile([128, 512], f32, tag="po2")
                    nc.tensor.matmul(
                        out=po2[:],
                        lhsT=mmcast(w4a_sb[:, hs]),
                        rhs=mmcast(o1h[0][:]),
                        start=True,
                        stop=False,
                    )
                    nc.tensor.matmul(
                        out=po2[:],
                        lhsT=mmcast(w4b_sb[:, hs]),
                        rhs=mmcast(o1h[1][:]),
                        start=False,
                        stop=True,
                    )
                    o2t = o2p.tile([128, 512], f32, tag="o2")
                    nc.scalar.activation(
                        out=o2t[:],
                        in_=po2[:],
                        func=mybir.ActivationFunctionType.Relu,
                        bias=b4_sb[:, h : h + 1],
                    )
                    nc.sync.dma_start(out=oT[hs, cols], in_=o2t[:])

    nc.finalize()
    return nc


# ---------------------------------------------------------------------------
# Entry point
# ---------------------------------------------------------------------------

def kernel(x, edge_index, edge_attr, W1, b1, W2, b2, W3, b3, W4, b4,
           edge_prec="bf16", node_prec="f32r"):
    x = np.asarray(x, dtype=np.float32)
    edge_index = np.asarray(edge_index)
    edge_attr = np.asarray(edge_attr, dtype=np.float32)
    W1 = np.asarray(W1, dtype=np.float32)
    b1 = np.asarray(b1, dtype=np.float32)
    W2 = np.asarray(W2, dtype=np.float32)
    b2 = np.asarray(b2, dtype=np.float32)
    W3 = np.asarray(W3, dtype=np.float32)
    b3 = np.asarray(b3, dtype=np.float32)
    W4 = np.asarray(W4, dtype=np.float32)
    b4 = np.asarray(b4, dtype=np.float32)

    row = np.asarray(edge_index[0], dtype=np.int64)
    col = np.asarray(edge_index[1], dtype=np.int64)

    edge_dt = ml_dtypes.bfloat16 if edge_prec == "bf16" else np.float32
    node_dt = ml_dtypes.bfloat16 if node_prec == "bf16" else np.float32
    f32r = node_prec == "f32r" or edge_prec == "f32r"

    in_maps, meta = _preprocess(x, row, col, edge_attr, edge_dt, node_dt)

    # weights: shared across cores
    w1c43 = np.vstack([W1[X_DIM:], W1[:X_DIM]])  # [43, 64]
    w1c = np.zeros((128, H1), dtype=np.float32)
    w1c[:EAX_DIM] = w1c43
    w1c[64 : 64 + EAX_DIM] = w1c43
    w1c = w1c.astype(edge_dt)
    w2d = np.vstack([W2, W2]).astype(edge_dt)  # [128, 128], one copy per pair half
    w3m = W3[X_DIM:].astype(node_dt)
    w3x = W3[:X_DIM].astype(node_dt)
    w4a = W4[:128].astype(node_dt)
    w4b = W4[128:].astype(node_dt)
    b1d = np.concatenate([b1, b1]).reshape(128, 1).astype(np.float32)
    b2d = b2.reshape(H2, 1).astype(np.float32)
    b3d = b3.reshape(2, 128).T.copy().astype(np.float32)
    b4d = b4.reshape(2, 128).T.copy().astype(np.float32)
    for m in in_maps:
        m.update(
            w1c=w1c, w2d=w2d, w3m=w3m, w3x=w3x, w4a=w4a, w4b=w4b,
            b1d=b1d, b2d=b2d, b3d=b3d, b4d=b4d,
        )

    nc = _build_program(
        meta, W1, b1, W2, b2, W3, b3, W4, b4,
        edge_dt, node_dt, matmul_f32r=f32r,
    )

    from concourse.bass_utils import run_bass_kernel_spmd

    res = run_bass_kernel_spmd(nc, in_maps, list(range(N_CORES)), **RUN_KWARGS)
    global LAST_EXEC_NS, LAST_RESULT
    LAST_EXEC_NS = res.exec_time_ns
    LAST_RESULT = res

    out = np.zeros((N_NODES, H3), dtype=np.float32)
    for c in range(N_CORES):
        oT_c = np.asarray(res.results[c]["oT"])  # [256, NS_pad]
        slots = meta["slot_tables"][c]
        valid = slots >= 0
        out[slots[valid]] = oT_c[:, valid].T
    return out


if __name__ == "__main__":
    # tiny self-test with a small synthetic graph via monkeypatched sizes
    pass
